# revision 34
# baseline (speedup 1.0000x reference)
"""Causal attention layer (N=8, L=2048, H=1024, E=64) on 8 TRN2 NeuronCores.

Sharding: data-parallel over batch N — one batch element per core, Q/K/V
projection weights replicated. No collectives needed.

Per-core pipeline (memory-bound problem: 24MB of q/k/v per core):
  1. q/k/v cast-loaded (f32 DRAM -> bf16 SBUF, SWDGE cast DMA) in 512-row
     chunks, then ONE flat XBAR-DMA-transpose per (tensor, chunk):
     in [128, 4096] -> out [128, 4096] whose free index m encodes
     (lp, lt, hb) = (m//32, (m%32)//8, m%8); the projection's moving-operand
     APs read it with strides [(lt:8), (lp:32)] at offset hb, which restores
     natural l-order in PSUM columns.
  2. Projections: stationary WqT/WkT/WvT [128, 64] blocks (xbar-transposed
     once), moving chunk stripes -> qpT/kpT/vpT [64, 2048] bf16, bias added
     on ScalarE during the PSUM->SBUF copy.
  3. vpT is PE-transposed to natural vp [128, 65] blocks with an appended
     ones-column (makes the context matmul accumulate softmax row-sums for
     free).
  4. Scores computed transposed: PT[j, i] = exp(scale * kp_j . qp_i), exp on
     ScalarE with the 1/sqrt(L) scale folded in; causal mask = multiplicative
     upper-triangular mask on diagonal blocks (scores are tiny: no
     max-subtraction needed).
  5. ctxT[65, i] += vp_aug[j].T @ PT[j, i] accumulated over j in PSUM;
     epilogue PE-transposes ctxT back to natural, divides by the row-sum
     column, DMAs out per stripe.
Loads are emitted k,v,q per chunk with q's last chunk hoisted before k/v's
last chunk so the deep final attention stripe starts before the load stream
finishes.
"""

import math

import numpy as np

N, L, H, E = 8, 2048, 1024, 64
NCORES = 8
CHUNK = 512  # rows per load chunk
NCHUNK = L // CHUNK  # 4
TPC = CHUNK // 128  # 128-row tiles per chunk = 4
NBLK = L // 128  # 16 j/i blocks
HB = H // 128  # 8 h-blocks

_CACHE = {}


def _build_nc(reps=1):
    from contextlib import ExitStack

    import concourse.mybir as mybir
    import concourse.tile as tile
    from concourse import bacc
    from concourse.masks import make_identity, make_upper_triangular

    f32 = mybir.dt.float32
    bf16 = mybir.dt.bfloat16
    AF = mybir.ActivationFunctionType
    scale = 1.0 / math.sqrt(float(L))

    nc = bacc.Bacc("TRN2", target_bir_lowering=False, debug=False)

    q_ap = nc.dram_tensor("q", [L, H], f32, kind="ExternalInput").ap()
    k_ap = nc.dram_tensor("k", [L, H], f32, kind="ExternalInput").ap()
    v_ap = nc.dram_tensor("v", [L, H], f32, kind="ExternalInput").ap()
    wq_ap = nc.dram_tensor("wq", [E, H], f32, kind="ExternalInput").ap()
    wk_ap = nc.dram_tensor("wk", [E, H], f32, kind="ExternalInput").ap()
    wv_ap = nc.dram_tensor("wv", [E, H], f32, kind="ExternalInput").ap()
    bq_ap = nc.dram_tensor("bq", [E], f32, kind="ExternalInput").ap()
    bk_ap = nc.dram_tensor("bk", [E], f32, kind="ExternalInput").ap()
    bv_ap = nc.dram_tensor("bv", [E], f32, kind="ExternalInput").ap()
    out_ap = nc.dram_tensor("out", [L, E], f32, kind="ExternalOutput").ap()

    with tile.TileContext(nc) as tc, ExitStack() as ctx:
        const = ctx.enter_context(tc.tile_pool(name="const", bufs=1))
        natp = ctx.enter_context(tc.tile_pool(name="nat", bufs=6))
        chp = ctx.enter_context(tc.tile_pool(name="ch", bufs=8))
        pTsb = ctx.enter_context(tc.tile_pool(name="pTsb", bufs=1))
        projps = ctx.enter_context(tc.tile_pool(name="projps", bufs=2, space="PSUM"))
        scps = ctx.enter_context(tc.tile_pool(name="scps", bufs=3, space="PSUM"))
        ptp = ctx.enter_context(tc.tile_pool(name="pt", bufs=3))
        ctxps = ctx.enter_context(tc.tile_pool(name="ctxps", bufs=2, space="PSUM"))
        tpsp = ctx.enter_context(tc.tile_pool(name="tps", bufs=1, space="PSUM"))
        epip = ctx.enter_context(tc.tile_pool(name="epi", bufs=2))

        # --- constants ---
        ident_f32 = const.tile([128, 128], f32)
        make_identity(nc, ident_f32[:])
        ident_bf16 = const.tile([128, 128], bf16)
        nc.vector.tensor_copy(ident_bf16[:], ident_f32[:])
        # tri[r, c] = 1.0 where c >= r (valid: key block-row <= query block-col)
        tri_f32 = const.tile([128, 128], f32)
        make_upper_triangular(nc, tri_f32[:], val=1.0, diag=True)
        tri = const.tile([128, 128], bf16)
        nc.vector.tensor_copy(tri[:], tri_f32[:])

        # --- weights: cast-load natural [64, 1024], xbar-transpose to
        # [128(h%128), 8(h//128), 64(e)]; biases [64, 1] ---
        wT = {}
        b_sb = {}
        for name, w_ap, bias_ap in (
            ("q", wq_ap, bq_ap),
            ("k", wk_ap, bk_ap),
            ("v", wv_ap, bv_ap),
        ):
            wnat = const.tile([E, H], bf16, tag=f"wnat_{name}")
            nc.gpsimd.dma_start(out=wnat[:], in_=w_ap)  # f32 -> bf16 cast
            wt = const.tile([128, HB, E], bf16, tag=f"wT_{name}")
            nc.sync.dma_start(out=wt[:], in_=wnat[:], transpose=True)
            wT[name] = wt
            bs = const.tile([E, 1], f32, tag=f"b_{name}")
            nc.scalar.dma_start(out=bs[:], in_=bias_ap)
            b_sb[name] = bs

        # --- persistent projection outputs ---
        qpT = pTsb.tile([E, L], bf16, tag="qpT")
        kpT = pTsb.tile([E, L], bf16, tag="kpT")
        vpT = pTsb.tile([E, L], bf16, tag="vpT")
        vaug = pTsb.tile([128, NBLK, E + 1], bf16, tag="vaug")
        nc.vector.memset(vaug[:, :, E : E + 1], 1.0)

        pT_of = {"q": qpT, "k": kpT, "v": vpT}
        x_ap_of = {"q": q_ap, "k": k_ap, "v": v_ap}

        def emit_load_and_proj(name, c):
            l0 = c * CHUNK
            nat = natp.tile([128, TPC, H], bf16, tag="nat")
            src = x_ap_of[name][l0 : l0 + CHUNK, :].rearrange(
                "(t p) h -> p t h", p=128
            )
            nc.gpsimd.dma_start(out=nat[:], in_=src)  # f32 -> bf16 cast
            cht = chp.tile([128, TPC * H], bf16, tag="ch")
            if name == "v":
                # transpose on PE (saves serial-DMA xbar time): per (lt, hb)
                # 128x128 block transpose into PSUM, evacuate per-hb to SBUF
                # vT chunk [128, hb, l]; evac alternates ScalarE/VectorE.
                chv = cht[:].rearrange("p (hb l) -> p hb l", hb=HB, l=CHUNK)
                for hb in range(HB):
                    vt_ps = scps.tile([128, CHUNK], bf16, tag="sc")
                    for t in range(TPC):
                        nc.tensor.transpose(
                            vt_ps[:, t * 128 : (t + 1) * 128],
                            nat[:, t, hb * 128 : (hb + 1) * 128],
                            ident_bf16[:],
                        )
                    if hb % 2 == 0:
                        nc.scalar.activation(
                            chv[:, hb, :], vt_ps[:], AF.Identity)
                    else:
                        nc.vector.tensor_copy(chv[:, hb, :], vt_ps[:])
                rhs_of = lambda hb: chv[:, hb, :]
            else:
                # ONE xbar transpose per chunk: 3D out [128, TPC*HB, 128]
                # with out[a, b, c] = nat_flat[c, b*128 + a] (3D-out form
                # validated against the execution backend); free layout is
                # t*1024 + hb*128 + lp, so the projection's moving-operand AP
                # [(t: 1024), (lp: 1)] at offset hb*128 is natural l-order.
                chb = cht[:].rearrange(
                    "p (t hb lp) -> p t hb lp", t=TPC, hb=HB, lp=128
                )
                nc.sync.dma_start(
                    out=cht[:].rearrange("p (b c) -> p b c", b=TPC * HB, c=128),
                    in_=nat[:].rearrange("p t h -> p (t h)"),
                    transpose=True,
                )
                rhs_of = lambda hb: chb[:, :, hb, :]
            ps = projps.tile([E, CHUNK], f32, tag="projps")
            for hb in range(HB):
                nc.tensor.matmul(
                    ps[:],
                    lhsT=wT[name][:, hb, :],
                    rhs=rhs_of(hb),
                    start=(hb == 0),
                    stop=(hb == HB - 1),
                )
            nc.scalar.activation(
                pT_of[name][:, l0 : l0 + CHUNK], ps[:], AF.Identity,
                bias=b_sb[name][:],
            )
            if name == "v":
                for t in range(TPC):
                    jb = c * TPC + t
                    vps = tpsp.tile([128, E + 1], bf16, tag="tps")
                    nc.tensor.transpose(
                        vps[:, :E],
                        vpT[:, jb * 128 : (jb + 1) * 128],
                        ident_bf16[:E, :E],
                    )
                    nc.vector.tensor_copy(vaug[:, jb, 0:E], vps[:, :E])

        def emit_stripe(s):
            i0 = s * CHUNK
            i1 = i0 + CHUNK
            jmax = (i1 // 128) - 1
            ctx_ps = ctxps.tile([E + 1, CHUNK], f32, tag="ctx")
            for j in range(jmax + 1):
                g0 = max(i0, j * 128)
                w = i1 - g0
                sc = scps.tile([128, CHUNK], f32, tag="sc")
                pt = ptp.tile([128, CHUNK], bf16, tag="pt")
                nc.tensor.matmul(
                    sc[:, 0:w],
                    lhsT=kpT[:, j * 128 : (j + 1) * 128],
                    rhs=qpT[:, g0 : g0 + w],
                    start=True,
                    stop=True,
                )
                nc.scalar.activation(pt[:, 0:w], sc[:, 0:w], AF.Exp, scale=scale)
                if g0 == j * 128:  # diagonal block: causal mask
                    nc.vector.tensor_mul(pt[:, 0:128], pt[:, 0:128], tri[:])
                nc.tensor.matmul(
                    ctx_ps[:, g0 - i0 : g0 - i0 + w],
                    lhsT=vaug[:, j, :],
                    rhs=pt[:, 0:w],
                    start=(j == 0),
                    stop=(j == jmax),
                )
            # epilogue: PSUM ctxT -> SBUF, transpose to natural, divide by rowsum
            ctxsb = epip.tile([E + 1, CHUNK], f32, tag="ctxsb")
            nc.scalar.activation(ctxsb[:], ctx_ps[:], AF.Identity)
            outsb = epip.tile([128, TPC, E], f32, tag="outsb")
            for t in range(TPC):
                cps = tpsp.tile([128, E + 1], f32, tag="tps")
                nc.tensor.transpose(
                    cps[:],
                    ctxsb[:, t * 128 : (t + 1) * 128],
                    ident_f32[: E + 1, : E + 1],
                )
                rec = epip.tile([128, 1], f32, tag="rec")
                nc.vector.reciprocal(rec[:], cps[:, E : E + 1])
                nc.vector.tensor_scalar_mul(outsb[:, t, :], cps[:, 0:E], rec[:])
            dst = out_ap[i0:i1, :].rearrange("(t p) e -> p t e", p=128)
            nc.scalar.dma_start(out=dst, in_=outsb[:])

        # Load order: (k,v,q) per chunk, q's last chunk hoisted before k/v's
        # last chunk; stripe s emitted once chunk s is fully emitted.
        load_order = []
        for c in range(NCHUNK - 1):
            load_order += [("k", c), ("v", c), ("q", c)]
        load_order += [("q", NCHUNK - 1), ("k", NCHUNK - 1), ("v", NCHUNK - 1)]

        for _ in range(reps):
            done = set()
            for name, c in load_order:
                emit_load_and_proj(name, c)
                done.add((name, c))
                if all((t, c) in done for t in ("q", "k", "v")):
                    emit_stripe(c)

    nc.compile()
    return nc


def _get_nc(reps=1):
    key = ("nc", reps)
    if key not in _CACHE:
        _CACHE[key] = _build_nc(reps)
    return _CACHE[key]


def kernel(q, k, v, key_padding_mask=None, Wq=None, bq=None, Wk=None, bk=None,
           Wv=None, bv=None):
    from concourse.bass_utils import run_bass_kernel_spmd

    nc = _get_nc()
    f = np.float32
    shared = {
        "wq": np.ascontiguousarray(Wq, dtype=f),
        "wk": np.ascontiguousarray(Wk, dtype=f),
        "wv": np.ascontiguousarray(Wv, dtype=f),
        "bq": np.ascontiguousarray(bq, dtype=f),
        "bk": np.ascontiguousarray(bk, dtype=f),
        "bv": np.ascontiguousarray(bv, dtype=f),
    }
    in_maps = []
    for n in range(NCORES):
        m = dict(shared)
        m["q"] = np.ascontiguousarray(q[n], dtype=f)
        m["k"] = np.ascontiguousarray(k[n], dtype=f)
        m["v"] = np.ascontiguousarray(v[n], dtype=f)
        in_maps.append(m)
    res = run_bass_kernel_spmd(nc, in_maps, core_ids=list(range(NCORES)))
    out = np.stack([res.results[i]["out"] for i in range(NCORES)], axis=0)
    return out.astype(np.float32)


# revision 65
# speedup vs baseline: 1.2262x; 1.2262x over previous
"""Causal attention layer (N=8, L=2048, H=1024, E=64) on 8 TRN2 NeuronCores.

Sharding: data-parallel over batch N — one batch element per core, Q/K/V
projection weights replicated. No collectives needed.

Per-core pipeline (memory-bound problem: 24MB of q/k/v per core):
  1. q/k/v cast-loaded (f32 DRAM -> bf16 SBUF, SWDGE cast DMA) in 512-row
     chunks, then ONE flat XBAR-DMA-transpose per (tensor, chunk):
     in [128, 4096] -> out [128, 4096] whose free index m encodes
     (lp, lt, hb) = (m//32, (m%32)//8, m%8); the projection's moving-operand
     APs read it with strides [(lt:8), (lp:32)] at offset hb, which restores
     natural l-order in PSUM columns.
  2. Projections: stationary WqT/WkT/WvT [128, 64] blocks (xbar-transposed
     once), moving chunk stripes -> qpT/kpT/vpT [64, 2048] bf16, bias added
     on ScalarE during the PSUM->SBUF copy.
  3. vpT is PE-transposed to natural vp [128, 65] blocks with an appended
     ones-column (makes the context matmul accumulate softmax row-sums for
     free).
  4. Scores computed transposed: PT[j, i] = exp(scale * kp_j . qp_i), exp on
     ScalarE with the 1/sqrt(L) scale folded in; causal mask = multiplicative
     upper-triangular mask on diagonal blocks (scores are tiny: no
     max-subtraction needed).
  5. ctxT[65, i] += vp_aug[j].T @ PT[j, i] accumulated over j in PSUM;
     epilogue PE-transposes ctxT back to natural, divides by the row-sum
     column, DMAs out per stripe.
Loads are emitted k,v,q per chunk with q's last chunk hoisted before k/v's
last chunk so the deep final attention stripe starts before the load stream
finishes.
"""

import math

import numpy as np

N, L, H, E = 8, 2048, 1024, 64
NCORES = 8
CHUNK = 512  # rows per load chunk
NCHUNK = L // CHUNK  # 4
TPC = CHUNK // 128  # 128-row tiles per chunk = 4
NBLK = L // 128  # 16 j/i blocks
HB = H // 128  # 8 h-blocks

_CACHE = {}


def _build_nc(reps=1):
    from contextlib import ExitStack

    import concourse.mybir as mybir
    import concourse.tile as tile
    from concourse import bacc
    from concourse.tile_rust import add_dep_helper
    from concourse.masks import make_identity, make_upper_triangular

    f32 = mybir.dt.float32
    bf16 = mybir.dt.bfloat16
    AF = mybir.ActivationFunctionType
    scale = 1.0 / math.sqrt(float(L))

    nc = bacc.Bacc("TRN2", target_bir_lowering=False, debug=False)

    q_ap = nc.dram_tensor("q", [L, H], f32, kind="ExternalInput").ap()
    k_ap = nc.dram_tensor("k", [L, H], f32, kind="ExternalInput").ap()
    v_ap = nc.dram_tensor("v", [L, H], f32, kind="ExternalInput").ap()
    wq_ap = nc.dram_tensor("wq", [E, H], f32, kind="ExternalInput").ap()
    wk_ap = nc.dram_tensor("wk", [E, H], f32, kind="ExternalInput").ap()
    wv_ap = nc.dram_tensor("wv", [E, H], f32, kind="ExternalInput").ap()
    bq_ap = nc.dram_tensor("bq", [E], f32, kind="ExternalInput").ap()
    bk_ap = nc.dram_tensor("bk", [E], f32, kind="ExternalInput").ap()
    bv_ap = nc.dram_tensor("bv", [E], f32, kind="ExternalInput").ap()
    out_ap = nc.dram_tensor("out", [L, E], f32, kind="ExternalOutput").ap()

    with tile.TileContext(nc) as tc, ExitStack() as ctx:
        const = ctx.enter_context(tc.tile_pool(name="const", bufs=1))
        natp = ctx.enter_context(tc.tile_pool(name="nat", bufs=9))
        chp = ctx.enter_context(tc.tile_pool(name="ch", bufs=8))
        pTsb = ctx.enter_context(tc.tile_pool(name="pTsb", bufs=1))
        projps = ctx.enter_context(tc.tile_pool(name="projps", bufs=2, space="PSUM"))
        scps = ctx.enter_context(tc.tile_pool(name="scps", bufs=3, space="PSUM"))
        ptp = ctx.enter_context(tc.tile_pool(name="pt", bufs=3))
        ctxps = ctx.enter_context(tc.tile_pool(name="ctxps", bufs=2, space="PSUM"))
        tpsp = ctx.enter_context(tc.tile_pool(name="tps", bufs=1, space="PSUM"))
        epip = ctx.enter_context(tc.tile_pool(name="epi", bufs=4))

        # --- constants & weights: emitted via a deferred hook after the
        # first big loads so they don't block the Pool DMA queue; W is
        # sync-loaded f32 (HWDGE) and cast on VectorE, then xbar-transposed
        # to [128(h%128), 8(h//128), 64(e)] ---
        ident_f32 = const.tile([128, 128], f32)
        ident_bf16 = const.tile([128, 128], bf16)
        tri_f32 = const.tile([128, 128], f32)
        tri = const.tile([128, 128], bf16)
        wT = {}
        b_sb = {}
        wnatf = {}
        for _n in ("q", "k", "v"):
            wnatf[_n] = const.tile([E, H], f32, tag=f"wnatf_{_n}",
                                   name=f"wnatf_{_n}")
            wT[_n] = const.tile([128, HB, E], bf16, tag=f"wT_{_n}",
                                name=f"wT_{_n}")
            b_sb[_n] = const.tile([E, 1], f32, tag=f"b_{_n}",
                                  name=f"b_{_n}")

        w_xbars = []

        def emit_consts_and_weights(vaug):
            nc.vector.memset(vaug[:, :, E : E + 1], 1.0)
            make_identity(nc, ident_f32[:])
            nc.vector.tensor_copy(ident_bf16[:], ident_f32[:])
            # tri[r, c] = 1.0 where c >= r (valid: key row <= query col)
            make_upper_triangular(nc, tri_f32[:], val=1.0, diag=True)
            nc.vector.tensor_copy(tri[:], tri_f32[:])
            for name, w_ap, bias_ap in (
                ("q", wq_ap, bq_ap),
                ("k", wk_ap, bk_ap),
                ("v", wv_ap, bv_ap),
            ):
                nc.sync.dma_start(out=wnatf[name][:], in_=w_ap)
                wnat = const.tile([E, H], bf16, tag=f"wnat_{name}")
                nc.vector.tensor_copy(wnat[:], wnatf[name][:])
                w_xbars.append(
                    nc.sync.dma_start(out=wT[name][:], in_=wnat[:],
                                      transpose=True))
                nc.scalar.dma_start(out=b_sb[name][:], in_=bias_ap)

        # --- persistent projection outputs ---
        qpT = pTsb.tile([E, L], bf16, tag="qpT")
        kpT = pTsb.tile([E, L], bf16, tag="kpT")
        vpT = pTsb.tile([E, L], bf16, tag="vpT")
        vaug = pTsb.tile([128, NBLK, E + 1], bf16, tag="vaug")

        pT_of = {"q": qpT, "k": kpT, "v": vpT}
        x_ap_of = {"q": q_ap, "k": k_ap, "v": v_ap}

        out_dmas = []

        def emit_load(name, c):
            l0 = c * CHUNK
            nat = natp.tile([128, TPC, H], bf16, tag="nat")
            src = x_ap_of[name][l0 : l0 + CHUNK, :].rearrange(
                "(t p) h -> p t h", p=128
            )
            # flat out AP: bigger contiguous runs -> half the SWDGE
            # descriptors, so more loads fit the descriptor ring at once
            ld = nc.gpsimd.dma_start(
                out=nat[:].rearrange("p t h -> p (t h)"), in_=src
            )  # f32 -> bf16 cast
            return nat, ld

        def emit_tp_and_proj(name, c, nat):
            l0 = c * CHUNK
            xb = None
            cht = chp.tile([128, TPC * H], bf16, tag="ch")
            if name == "v":
                # transpose on PE (saves serial-DMA xbar time): per (lt, hb)
                # 128x128 block transpose into PSUM, evacuate per-hb to SBUF
                # vT chunk [128, hb, l]; evac alternates ScalarE/VectorE.
                chv = cht[:].rearrange("p (hb l) -> p hb l", hb=HB, l=CHUNK)
                for hb in range(HB):
                    vt_ps = scps.tile([128, CHUNK], bf16, tag="sc")
                    for t in range(TPC):
                        nc.tensor.transpose(
                            vt_ps[:, t * 128 : (t + 1) * 128],
                            nat[:, t, hb * 128 : (hb + 1) * 128],
                            ident_bf16[:],
                        )
                    if hb % 2 == 0:
                        nc.scalar.activation(
                            chv[:, hb, :], vt_ps[:], AF.Identity)
                    else:
                        nc.vector.tensor_copy(chv[:, hb, :], vt_ps[:])
                rhs_of = lambda hb: chv[:, hb, :]
            else:
                # ONE xbar transpose per chunk: 3D out [128, TPC*HB, 128]
                # with out[a, b, c] = nat_flat[c, b*128 + a] (3D-out form
                # validated against the execution backend); free layout is
                # t*1024 + hb*128 + lp, so the projection's moving-operand AP
                # [(t: 1024), (lp: 1)] at offset hb*128 is natural l-order.
                chb = cht[:].rearrange(
                    "p (t hb lp) -> p t hb lp", t=TPC, hb=HB, lp=128
                )
                xb = nc.sync.dma_start(
                    out=cht[:].rearrange("p (b c) -> p b c", b=TPC * HB, c=128),
                    in_=nat[:].rearrange("p t h -> p (t h)"),
                    transpose=True,
                )
                rhs_of = lambda hb: chb[:, :, hb, :]
            ps = projps.tile([E, CHUNK], f32, tag="projps")
            for hb in range(HB):
                nc.tensor.matmul(
                    ps[:],
                    lhsT=wT[name][:, hb, :],
                    rhs=rhs_of(hb),
                    start=(hb == 0),
                    stop=(hb == HB - 1),
                )
            nc.scalar.activation(
                pT_of[name][:, l0 : l0 + CHUNK], ps[:], AF.Identity,
                bias=b_sb[name][:],
            )
            if name == "v":
                for t in range(TPC):
                    jb = c * TPC + t
                    vps = tpsp.tile([128, E + 1], bf16, tag="tps")
                    nc.tensor.transpose(
                        vps[:, :E],
                        vpT[:, jb * 128 : (jb + 1) * 128],
                        ident_bf16[:E, :E],
                    )
                    nc.vector.tensor_copy(vaug[:, jb, 0:E], vps[:, :E])
            return xb

        def begin_stripe(s):
            ctx_ps = ctxps.tile([E + 1, CHUNK], f32, tag="ctx")
            return {"s": s, "ctx": ctx_ps, "jmax": (s + 1) * TPC - 1}

        def emit_js(st, js):
            s, ctx_ps, jmax = st["s"], st["ctx"], st["jmax"]
            i0, i1 = s * CHUNK, (s + 1) * CHUNK
            # pair adjacent j's so exp runs on wider tiles (one PSUM bank)
            js = list(js)
            pairs = []
            while js:
                take = js[:1]
                w0 = i1 - max(i0, js[0] * 128)
                if len(js) > 1 and w0 + (i1 - max(i0, js[1] * 128)) <= 512:
                    take = js[:2]
                pairs.append(take)
                js = js[len(take):]
            def emit_ctx(pt, infos):
                for j, g0, w, o in infos:
                    if g0 == j * 128:  # diagonal block: causal mask
                        nc.vector.tensor_mul(
                            pt[:, o : o + 128], pt[:, o : o + 128], tri[:]
                        )
                    nc.tensor.matmul(
                        ctx_ps[:, g0 - i0 : g0 - i0 + w],
                        lhsT=vaug[:, j, :],
                        rhs=pt[:, o : o + w],
                        start=(j == 0),
                        stop=(j == jmax),
                    )

            # one-group software skew: PE's in-order queue sees
            # [scores_p, ctx_{p-1}] so it never stalls on exp_p
            pending = None
            for take in pairs:
                sc = scps.tile([128, 512], f32, tag="sc")
                pt = ptp.tile([128, 512], bf16, tag="pt")
                infos = []
                off = 0
                for j in take:
                    g0 = max(i0, j * 128)
                    w = i1 - g0
                    nc.tensor.matmul(
                        sc[:, off : off + w],
                        lhsT=kpT[:, j * 128 : (j + 1) * 128],
                        rhs=qpT[:, g0 : g0 + w],
                        start=True,
                        stop=True,
                    )
                    infos.append((j, g0, w, off))
                    off += w
                nc.scalar.activation(pt[:, 0:off], sc[:, 0:off], AF.Exp,
                                     scale=scale)
                if pending is not None:
                    emit_ctx(*pending)
                pending = (pt, infos)
            emit_ctx(*pending)

        def end_stripe(st):
            s, ctx_ps = st["s"], st["ctx"]
            i0, i1 = s * CHUNK, (s + 1) * CHUNK
            ctxsb = epip.tile([E + 1, CHUNK], f32, tag="ctxsb")
            nc.vector.tensor_copy(ctxsb[:], ctx_ps[:])
            outsb = epip.tile([128, TPC, E], f32, tag="outsb")
            for t in range(TPC):
                cps = tpsp.tile([128, E + 1], f32, tag="tps")
                nc.tensor.transpose(
                    cps[:],
                    ctxsb[:, t * 128 : (t + 1) * 128],
                    ident_f32[: E + 1, : E + 1],
                )
                rec = epip.tile([128, 1], f32, tag="rec")
                nc.vector.reciprocal(rec[:], cps[:, E : E + 1])
                nc.vector.tensor_scalar_mul(outsb[:, t, :], cps[:, 0:E], rec[:])
            dst = out_ap[i0:i1, :].rearrange("(t p) e -> p t e", p=128)
            out_dmas.append((dst, outsb))

        # Load order: (k,v,q) per chunk, q's last chunk hoisted before k/v's
        # last chunk; stripe s emitted once chunk s is fully emitted.
        load_order = []
        for c in range(NCHUNK - 1):
            load_order += [("k", c), ("v", c), ("q", c)]
        load_order += [("q", NCHUNK - 1), ("k", NCHUNK - 1), ("v", NCHUNK - 1)]

        consts_done = [False]
        for _ in range(reps):
            # Tile globally serializes every DMACopy<->DmaTranspose mode
            # transition (~2.5us dead DMA time each), so batch the stream
            # into phases: [6 loads][4 xbar transposes][6 loads][4 xbars]
            # [4 output copies] - 5 transitions instead of ~11.
            out_dmas.clear()
            # schedule: phase lists of loads; after each load's tp_proj,
            # run the stripe actions keyed to that (tensor, chunk).
            # Stripe-3's v-independent j's are emitted right after q3's
            # projection so they overlap the remaining loads; its
            # v3-dependent tail (j 12-15) comes after v3's vaug blocks.
            st_of = {}
            def s_begin(c):
                st_of[c] = begin_stripe(c)
            def s_js(c, js):
                emit_js(st_of[c], js)
            def s_end(c):
                end_stripe(st_of[c])
            phases = [
                (("k", 0), ("v", 0),
                 ("q", 0, lambda: (s_begin(0), s_js(0, range(4)), s_end(0))),
                 ("k", 1), ("v", 1),
                 ("q", 1, lambda: (s_begin(1), s_js(1, range(8)), s_end(1))),
                 ("q", 3, lambda: (s_begin(3), s_js(3, range(8))))),
                (("k", 2),
                 ("v", 2, lambda: s_js(3, range(8, 12))),
                 ("q", 2, lambda: (s_begin(2), s_js(2, range(12)), s_end(2))),
                 ("k", 3),
                 ("v", 3, lambda: (s_js(3, range(12, 16)), s_end(3)))),
            ]
            prev_last_xb = None
            for phase in phases:
                nats = []
                last_ld = None
                for item in phase:
                    n, c = item[0], item[1]
                    nat, ld = emit_load(n, c)
                    if prev_last_xb is not None:
                        add_dep_helper(
                            ld.ins, prev_last_xb.ins, sync=True,
                            reason="dma mode-phase grouping: loads after "
                                   "previous phase's transposes")
                    nats.append((item, nat))
                    last_ld = ld
                xbs = []
                if not consts_done[0]:
                    consts_done[0] = True
                    emit_consts_and_weights(vaug)
                    for wxb in w_xbars:
                        add_dep_helper(
                            wxb.ins, last_ld.ins, sync=True,
                            reason="dma mode-phase grouping: W transposes "
                                   "with first xbar group")
                for item, nat in nats:
                    n, c = item[0], item[1]
                    xb = emit_tp_and_proj(n, c, nat)
                    if xb is not None:
                        add_dep_helper(
                            xb.ins, last_ld.ins, sync=True,
                            reason="dma mode-phase grouping: transposes "
                                   "after all phase loads")
                        xbs.append(xb)
                    if len(item) > 2:
                        item[2]()
                if xbs:
                    prev_last_xb = xbs[-1]
            for dst, outsb in out_dmas:
                od = nc.scalar.dma_start(out=dst, in_=outsb[:])
                add_dep_helper(
                    od.ins, prev_last_xb.ins, sync=True,
                    reason="dma mode-phase grouping: outputs last")

    nc.compile()
    return nc


def _get_nc(reps=1):
    key = ("nc", reps)
    if key not in _CACHE:
        _CACHE[key] = _build_nc(reps)
    return _CACHE[key]


def kernel(q, k, v, key_padding_mask=None, Wq=None, bq=None, Wk=None, bk=None,
           Wv=None, bv=None):
    from concourse.bass_utils import run_bass_kernel_spmd

    nc = _get_nc()
    f = np.float32
    shared = {
        "wq": np.ascontiguousarray(Wq, dtype=f),
        "wk": np.ascontiguousarray(Wk, dtype=f),
        "wv": np.ascontiguousarray(Wv, dtype=f),
        "bq": np.ascontiguousarray(bq, dtype=f),
        "bk": np.ascontiguousarray(bk, dtype=f),
        "bv": np.ascontiguousarray(bv, dtype=f),
    }
    in_maps = []
    for n in range(NCORES):
        m = dict(shared)
        m["q"] = np.ascontiguousarray(q[n], dtype=f)
        m["k"] = np.ascontiguousarray(k[n], dtype=f)
        m["v"] = np.ascontiguousarray(v[n], dtype=f)
        in_maps.append(m)
    res = run_bass_kernel_spmd(nc, in_maps, core_ids=list(range(NCORES)))
    out = np.stack([res.results[i]["out"] for i in range(NCORES)], axis=0)
    return out.astype(np.float32)


# revision 73
# speedup vs baseline: 1.2683x; 1.0343x over previous
"""Causal attention layer (N=8, L=2048, H=1024, E=64) on 8 TRN2 NeuronCores.

Sharding: data-parallel over batch N — one batch element per core, Q/K/V
projection weights replicated. No collectives needed.

Per-core pipeline (memory-bound problem: 24MB of q/k/v per core):
  1. q/k/v cast-loaded (f32 DRAM -> bf16 SBUF, SWDGE cast DMA) in 512-row
     chunks, then ONE flat XBAR-DMA-transpose per (tensor, chunk):
     in [128, 4096] -> out [128, 4096] whose free index m encodes
     (lp, lt, hb) = (m//32, (m%32)//8, m%8); the projection's moving-operand
     APs read it with strides [(lt:8), (lp:32)] at offset hb, which restores
     natural l-order in PSUM columns.
  2. Projections: stationary WqT/WkT/WvT [128, 64] blocks (xbar-transposed
     once), moving chunk stripes -> qpT/kpT/vpT [64, 2048] bf16, bias added
     on ScalarE during the PSUM->SBUF copy.
  3. vpT is PE-transposed to natural vp [128, 65] blocks with an appended
     ones-column (makes the context matmul accumulate softmax row-sums for
     free).
  4. Scores computed transposed: PT[j, i] = exp(scale * kp_j . qp_i), exp on
     ScalarE with the 1/sqrt(L) scale folded in; causal mask = multiplicative
     upper-triangular mask on diagonal blocks (scores are tiny: no
     max-subtraction needed).
  5. ctxT[65, i] += vp_aug[j].T @ PT[j, i] accumulated over j in PSUM;
     epilogue PE-transposes ctxT back to natural, divides by the row-sum
     column, DMAs out per stripe.
Loads are emitted k,v,q per chunk with q's last chunk hoisted before k/v's
last chunk so the deep final attention stripe starts before the load stream
finishes.
"""

import math

import numpy as np

N, L, H, E = 8, 2048, 1024, 64
NCORES = 8
CHUNK = 512  # rows per load chunk
NCHUNK = L // CHUNK  # 4
TPC = CHUNK // 128  # 128-row tiles per chunk = 4
NBLK = L // 128  # 16 j/i blocks
HB = H // 128  # 8 h-blocks

_CACHE = {}


def _build_nc(reps=1):
    from contextlib import ExitStack

    import concourse.mybir as mybir
    import concourse.tile as tile
    from concourse import bacc
    from concourse.tile_rust import add_dep_helper
    from concourse.masks import make_identity, make_upper_triangular

    f32 = mybir.dt.float32
    bf16 = mybir.dt.bfloat16
    fp8 = mybir.dt.float8e4
    AF = mybir.ActivationFunctionType
    scale = 1.0 / math.sqrt(float(L))

    nc = bacc.Bacc("TRN2", target_bir_lowering=False, debug=False)

    q_ap = nc.dram_tensor("q", [L, H], f32, kind="ExternalInput").ap()
    k_ap = nc.dram_tensor("k", [L, H], f32, kind="ExternalInput").ap()
    v_ap = nc.dram_tensor("v", [L, H], f32, kind="ExternalInput").ap()
    wq_ap = nc.dram_tensor("wq", [E, H], f32, kind="ExternalInput").ap()
    wk_ap = nc.dram_tensor("wk", [E, H], f32, kind="ExternalInput").ap()
    wv_ap = nc.dram_tensor("wv", [E, H], f32, kind="ExternalInput").ap()
    bq_ap = nc.dram_tensor("bq", [E], f32, kind="ExternalInput").ap()
    bk_ap = nc.dram_tensor("bk", [E], f32, kind="ExternalInput").ap()
    bv_ap = nc.dram_tensor("bv", [E], f32, kind="ExternalInput").ap()
    out_ap = nc.dram_tensor("out", [L, E], f32, kind="ExternalOutput").ap()

    with tile.TileContext(nc) as tc, ExitStack() as ctx:
        const = ctx.enter_context(tc.tile_pool(name="const", bufs=1))
        natp = ctx.enter_context(tc.tile_pool(name="nat", bufs=9))
        chp = ctx.enter_context(tc.tile_pool(name="ch", bufs=8))
        pTsb = ctx.enter_context(tc.tile_pool(name="pTsb", bufs=1))
        projps = ctx.enter_context(tc.tile_pool(name="projps", bufs=1, space="PSUM"))
        scps = ctx.enter_context(tc.tile_pool(name="scps", bufs=2, space="PSUM"))
        ktps = ctx.enter_context(tc.tile_pool(name="ktps", bufs=2, space="PSUM"))
        ptp = ctx.enter_context(tc.tile_pool(name="pt", bufs=3))
        ctxps = ctx.enter_context(tc.tile_pool(name="ctxps", bufs=2, space="PSUM"))
        tpsp = ctx.enter_context(tc.tile_pool(name="tps", bufs=1, space="PSUM"))
        epip = ctx.enter_context(tc.tile_pool(name="epi", bufs=4))

        # --- constants & weights: emitted via a deferred hook after the
        # first big loads so they don't block the Pool DMA queue; W is
        # sync-loaded f32 (HWDGE) and cast on VectorE, then xbar-transposed
        # to [128(h%128), 8(h//128), 64(e)] ---
        ident_f32 = const.tile([128, 128], f32)
        ident_bf16 = const.tile([128, 128], bf16)
        ident_fp8 = const.tile([128, 128], fp8)
        wtk8 = const.tile([128, HB, E], fp8)
        tri_f32 = const.tile([128, 128], f32)
        tri = const.tile([128, 128], bf16)
        wT = {}
        b_sb = {}
        wnatf = {}
        for _n in ("q", "k", "v"):
            wnatf[_n] = const.tile([E, H], f32, tag=f"wnatf_{_n}",
                                   name=f"wnatf_{_n}")
            wT[_n] = const.tile([128, HB, E], bf16, tag=f"wT_{_n}",
                                name=f"wT_{_n}")
            b_sb[_n] = const.tile([E, 1], f32, tag=f"b_{_n}",
                                  name=f"b_{_n}")

        w_xbars = []

        def emit_consts_and_weights(vaug):
            nc.vector.memset(vaug[:, :, E : E + 1], 1.0)
            make_identity(nc, ident_f32[:])
            nc.vector.tensor_copy(ident_bf16[:], ident_f32[:])
            nc.vector.tensor_copy(ident_fp8[:], ident_f32[:])
            # tri[r, c] = 1.0 where c >= r (valid: key row <= query col)
            make_upper_triangular(nc, tri_f32[:], val=1.0, diag=True)
            nc.vector.tensor_copy(tri[:], tri_f32[:])
            for name, w_ap, bias_ap in (
                ("q", wq_ap, bq_ap),
                ("k", wk_ap, bk_ap),
                ("v", wv_ap, bv_ap),
            ):
                nc.sync.dma_start(out=wnatf[name][:], in_=w_ap)
                wnat = const.tile([E, H], bf16, tag=f"wnat_{name}")
                nc.vector.tensor_copy(wnat[:], wnatf[name][:])
                w_xbars.append(
                    nc.sync.dma_start(out=wT[name][:], in_=wnat[:],
                                      transpose=True))
                nc.scalar.dma_start(out=b_sb[name][:], in_=bias_ap)
            nc.vector.tensor_copy(wtk8[:], wT["k"][:])

        # --- persistent projection outputs ---
        qpT = pTsb.tile([E, L], bf16, tag="qpT")
        kpT = pTsb.tile([E, L], bf16, tag="kpT")
        vpT = pTsb.tile([E, L], bf16, tag="vpT")
        vaug = pTsb.tile([128, NBLK, E + 1], bf16, tag="vaug")

        pT_of = {"q": qpT, "k": kpT, "v": vpT}
        x_ap_of = {"q": q_ap, "k": k_ap, "v": v_ap}

        out_dmas = []

        def emit_load(name, c):
            l0 = c * CHUNK
            nat = natp.tile([128, TPC, H], bf16, tag="nat")
            src = x_ap_of[name][l0 : l0 + CHUNK, :].rearrange(
                "(t p) h -> p t h", p=128
            )
            # flat out AP: bigger contiguous runs -> half the SWDGE
            # descriptors, so more loads fit the descriptor ring at once
            ld = nc.gpsimd.dma_start(
                out=nat[:].rearrange("p t h -> p (t h)"), in_=src
            )  # f32 -> bf16 cast
            return nat, ld

        def emit_tp_and_proj(name, c, nat):
            l0 = c * CHUNK
            xb = None
            cht = chp.tile([128, TPC * H], bf16, tag="ch")
            if name == "k":
                # transpose on PE (saves serial-DMA xbar time): per (lt, hb)
                # 128x128 block transpose into PSUM, evacuate per-hb to SBUF
                # vT chunk [128, hb, l]; evac alternates ScalarE/VectorE.
                chv = cht[:].rearrange("p (hb l) -> p hb l", hb=HB, l=CHUNK)
                for hb in range(HB):
                    vt_ps = ktps.tile([128, CHUNK], bf16, tag="kt")
                    for t in range(TPC):
                        nc.tensor.transpose(
                            vt_ps[:, t * 128 : (t + 1) * 128],
                            nat[:, t, hb * 128 : (hb + 1) * 128],
                            ident_bf16[:],
                        )
                    if hb % 2 == 1:
                        nc.scalar.activation(
                            chv[:, hb, :], vt_ps[:], AF.Identity)
                    else:
                        nc.vector.tensor_copy(chv[:, hb, :], vt_ps[:])
                rhs_of = lambda hb: chv[:, hb, :]
                w_st = wT[name]
            else:
                # ONE xbar transpose per chunk: 3D out [128, TPC*HB, 128]
                # with out[a, b, c] = nat_flat[c, b*128 + a] (3D-out form
                # validated against the execution backend); free layout is
                # t*1024 + hb*128 + lp, so the projection's moving-operand AP
                # [(t: 1024), (lp: 1)] at offset hb*128 is natural l-order.
                chb = cht[:].rearrange(
                    "p (t hb lp) -> p t hb lp", t=TPC, hb=HB, lp=128
                )
                xb = nc.sync.dma_start(
                    out=cht[:].rearrange("p (b c) -> p b c", b=TPC * HB, c=128),
                    in_=nat[:].rearrange("p t h -> p (t h)"),
                    transpose=True,
                )
                rhs_of = lambda hb: chb[:, :, hb, :]
                w_st = wT[name]
            ps = projps.tile([E, CHUNK], f32, tag="projps")
            for hb in range(HB):
                nc.tensor.matmul(
                    ps[:],
                    lhsT=w_st[:, hb, :],
                    rhs=rhs_of(hb),
                    start=(hb == 0),
                    stop=(hb == HB - 1),
                )
            if name == "q":
                # VectorE is lighter-loaded than ScalarE here, and q's
                # projection gates each stripe's scores
                nc.vector.tensor_scalar_add(
                    pT_of[name][:, l0 : l0 + CHUNK], ps[:], b_sb[name][:])
            else:
                nc.scalar.activation(
                    pT_of[name][:, l0 : l0 + CHUNK], ps[:], AF.Identity,
                    bias=b_sb[name][:],
                )
            if name == "v":
                for t in range(TPC):
                    jb = c * TPC + t
                    vps = tpsp.tile([128, E + 1], bf16, tag="tps")
                    nc.tensor.transpose(
                        vps[:, :E],
                        vpT[:, jb * 128 : (jb + 1) * 128],
                        ident_bf16[:E, :E],
                    )
                    nc.vector.tensor_copy(vaug[:, jb, 0:E], vps[:, :E])
            return xb

        def begin_stripe(s):
            ctx_ps = ctxps.tile([E + 1, CHUNK], f32, tag="ctx")
            return {"s": s, "ctx": ctx_ps, "jmax": (s + 1) * TPC - 1}

        def emit_js(st, js):
            s, ctx_ps, jmax = st["s"], st["ctx"], st["jmax"]
            i0, i1 = s * CHUNK, (s + 1) * CHUNK
            # pair adjacent j's so exp runs on wider tiles (one PSUM bank)
            js = list(js)
            pairs = []
            while js:
                take = js[:1]
                w0 = i1 - max(i0, js[0] * 128)
                if len(js) > 1 and w0 + (i1 - max(i0, js[1] * 128)) <= 512:
                    take = js[:2]
                pairs.append(take)
                js = js[len(take):]
            def emit_ctx(pt, infos):
                for j, g0, w, o in infos:
                    if g0 == j * 128:  # diagonal block: causal mask
                        nc.vector.tensor_mul(
                            pt[:, o : o + 128], pt[:, o : o + 128], tri[:]
                        )
                    nc.tensor.matmul(
                        ctx_ps[:, g0 - i0 : g0 - i0 + w],
                        lhsT=vaug[:, j, :],
                        rhs=pt[:, o : o + w],
                        start=(j == 0),
                        stop=(j == jmax),
                    )

            # one-group software skew: PE's in-order queue sees
            # [scores_p, ctx_{p-1}] so it never stalls on exp_p
            pending = None
            for take in pairs:
                sc = scps.tile([128, 512], f32, tag="sc")
                pt = ptp.tile([128, 512], bf16, tag="pt")
                infos = []
                off = 0
                for j in take:
                    g0 = max(i0, j * 128)
                    w = i1 - g0
                    nc.tensor.matmul(
                        sc[:, off : off + w],
                        lhsT=kpT[:, j * 128 : (j + 1) * 128],
                        rhs=qpT[:, g0 : g0 + w],
                        start=True,
                        stop=True,
                    )
                    infos.append((j, g0, w, off))
                    off += w
                nc.scalar.activation(pt[:, 0:off], sc[:, 0:off], AF.Exp,
                                     scale=scale)
                if pending is not None:
                    emit_ctx(*pending)
                pending = (pt, infos)
            emit_ctx(*pending)

        def end_stripe(st):
            s, ctx_ps = st["s"], st["ctx"]
            i0, i1 = s * CHUNK, (s + 1) * CHUNK
            ctxsb = epip.tile([E + 1, CHUNK], f32, tag="ctxsb")
            nc.vector.tensor_copy(ctxsb[:], ctx_ps[:])
            outsb = epip.tile([128, TPC, E], f32, tag="outsb")
            for t in range(TPC):
                cps = tpsp.tile([128, E + 1], f32, tag="tps")
                nc.tensor.transpose(
                    cps[:],
                    ctxsb[:, t * 128 : (t + 1) * 128],
                    ident_f32[: E + 1, : E + 1],
                )
                rec = epip.tile([128, 1], f32, tag="rec")
                nc.vector.reciprocal(rec[:], cps[:, E : E + 1])
                nc.vector.tensor_scalar_mul(outsb[:, t, :], cps[:, 0:E], rec[:])
            dst = out_ap[i0:i1, :].rearrange("(t p) e -> p t e", p=128)
            out_dmas.append((dst, outsb))

        # Load order: (k,v,q) per chunk, q's last chunk hoisted before k/v's
        # last chunk; stripe s emitted once chunk s is fully emitted.
        load_order = []
        for c in range(NCHUNK - 1):
            load_order += [("k", c), ("v", c), ("q", c)]
        load_order += [("q", NCHUNK - 1), ("k", NCHUNK - 1), ("v", NCHUNK - 1)]

        consts_done = [False]
        for _ in range(reps):
            # Tile globally serializes every DMACopy<->DmaTranspose mode
            # transition (~2.5us dead DMA time each), so batch the stream
            # into phases: [6 loads][4 xbar transposes][6 loads][4 xbars]
            # [4 output copies] - 5 transitions instead of ~11.
            out_dmas.clear()
            # schedule: phase lists of loads; after each load's tp_proj,
            # run the stripe actions keyed to that (tensor, chunk).
            # Stripe-3's v-independent j's are emitted right after q3's
            # projection so they overlap the remaining loads; its
            # v3-dependent tail (j 12-15) comes after v3's vaug blocks.
            st_of = {}
            def s_begin(c):
                st_of[c] = begin_stripe(c)
            def s_js(c, js):
                emit_js(st_of[c], js)
            def s_end(c):
                end_stripe(st_of[c])
            phases = [
                (("k", 0), ("v", 0),
                 ("q", 0, lambda: (s_begin(0), s_js(0, range(4)), s_end(0))),
                 ("k", 1), ("v", 1),
                 ("q", 1, lambda: (s_begin(1), s_js(1, range(8)), s_end(1))),
                 ("q", 3, lambda: (s_begin(3), s_js(3, range(8))))),
                (("k", 2),
                 ("v", 2, lambda: s_js(3, range(8, 12))),
                 ("q", 2, lambda: (s_begin(2), s_js(2, range(12)), s_end(2))),
                 ("k", 3),
                 ("v", 3, lambda: (s_js(3, range(12, 16)), s_end(3)))),
            ]
            prev_last_xb = None
            for phase in phases:
                nats = []
                last_ld = None
                for item in phase:
                    n, c = item[0], item[1]
                    nat, ld = emit_load(n, c)
                    if prev_last_xb is not None:
                        add_dep_helper(
                            ld.ins, prev_last_xb.ins, sync=True,
                            reason="dma mode-phase grouping: loads after "
                                   "previous phase's transposes")
                    nats.append((item, nat))
                    last_ld = ld
                xbs = []
                if not consts_done[0]:
                    consts_done[0] = True
                    emit_consts_and_weights(vaug)
                    for wxb in w_xbars:
                        add_dep_helper(
                            wxb.ins, last_ld.ins, sync=True,
                            reason="dma mode-phase grouping: W transposes "
                                   "with first xbar group")
                for item, nat in nats:
                    n, c = item[0], item[1]
                    xb = emit_tp_and_proj(n, c, nat)
                    if xb is not None:
                        add_dep_helper(
                            xb.ins, last_ld.ins, sync=True,
                            reason="dma mode-phase grouping: transposes "
                                   "after all phase loads")
                        xbs.append(xb)
                    if len(item) > 2:
                        item[2]()
                if xbs:
                    prev_last_xb = xbs[-1]
            for dst, outsb in out_dmas:
                od = nc.scalar.dma_start(out=dst, in_=outsb[:])
                add_dep_helper(
                    od.ins, prev_last_xb.ins, sync=True,
                    reason="dma mode-phase grouping: outputs last")

    nc.compile()
    return nc


def _get_nc(reps=1):
    key = ("nc", reps)
    if key not in _CACHE:
        _CACHE[key] = _build_nc(reps)
    return _CACHE[key]


def kernel(q, k, v, key_padding_mask=None, Wq=None, bq=None, Wk=None, bk=None,
           Wv=None, bv=None):
    from concourse.bass_utils import run_bass_kernel_spmd

    nc = _get_nc()
    f = np.float32
    shared = {
        "wq": np.ascontiguousarray(Wq, dtype=f),
        "wk": np.ascontiguousarray(Wk, dtype=f),
        "wv": np.ascontiguousarray(Wv, dtype=f),
        "bq": np.ascontiguousarray(bq, dtype=f),
        "bk": np.ascontiguousarray(bk, dtype=f),
        "bv": np.ascontiguousarray(bv, dtype=f),
    }
    in_maps = []
    for n in range(NCORES):
        m = dict(shared)
        m["q"] = np.ascontiguousarray(q[n], dtype=f)
        m["k"] = np.ascontiguousarray(k[n], dtype=f)
        m["v"] = np.ascontiguousarray(v[n], dtype=f)
        in_maps.append(m)
    res = run_bass_kernel_spmd(nc, in_maps, core_ids=list(range(NCORES)))
    out = np.stack([res.results[i]["out"] for i in range(NCORES)], axis=0)
    return out.astype(np.float32)


# revision 74
# speedup vs baseline: 1.3291x; 1.0479x over previous
"""Causal attention layer (N=8, L=2048, H=1024, E=64) on 8 TRN2 NeuronCores.

Sharding: data-parallel over batch N — one batch element per core, Q/K/V
projection weights replicated. No collectives needed.

Per-core pipeline (memory-bound problem: 24MB of q/k/v per core):
  1. q/k/v cast-loaded (f32 DRAM -> bf16 SBUF, SWDGE cast DMA) in 512-row
     chunks, then ONE flat XBAR-DMA-transpose per (tensor, chunk):
     in [128, 4096] -> out [128, 4096] whose free index m encodes
     (lp, lt, hb) = (m//32, (m%32)//8, m%8); the projection's moving-operand
     APs read it with strides [(lt:8), (lp:32)] at offset hb, which restores
     natural l-order in PSUM columns.
  2. Projections: stationary WqT/WkT/WvT [128, 64] blocks (xbar-transposed
     once), moving chunk stripes -> qpT/kpT/vpT [64, 2048] bf16, bias added
     on ScalarE during the PSUM->SBUF copy.
  3. vpT is PE-transposed to natural vp [128, 65] blocks with an appended
     ones-column (makes the context matmul accumulate softmax row-sums for
     free).
  4. Scores computed transposed: PT[j, i] = exp(scale * kp_j . qp_i), exp on
     ScalarE with the 1/sqrt(L) scale folded in; causal mask = multiplicative
     upper-triangular mask on diagonal blocks (scores are tiny: no
     max-subtraction needed).
  5. ctxT[65, i] += vp_aug[j].T @ PT[j, i] accumulated over j in PSUM;
     epilogue PE-transposes ctxT back to natural, divides by the row-sum
     column, DMAs out per stripe.
Loads are emitted k,v,q per chunk with q's last chunk hoisted before k/v's
last chunk so the deep final attention stripe starts before the load stream
finishes.
"""

import math

import numpy as np

N, L, H, E = 8, 2048, 1024, 64
NCORES = 8
CHUNK = 512  # rows per load chunk
NCHUNK = L // CHUNK  # 4
TPC = CHUNK // 128  # 128-row tiles per chunk = 4
NBLK = L // 128  # 16 j/i blocks
HB = H // 128  # 8 h-blocks

_CACHE = {}


def _build_nc(reps=1):
    from contextlib import ExitStack

    import concourse.mybir as mybir
    import concourse.tile as tile
    from concourse import bacc
    from concourse.tile_rust import add_dep_helper
    from concourse.masks import make_identity, make_upper_triangular

    f32 = mybir.dt.float32
    bf16 = mybir.dt.bfloat16
    fp8 = mybir.dt.float8e4
    AF = mybir.ActivationFunctionType
    scale = 1.0 / math.sqrt(float(L))

    nc = bacc.Bacc("TRN2", target_bir_lowering=False, debug=False)

    q_ap = nc.dram_tensor("q", [L, H], f32, kind="ExternalInput").ap()
    k_ap = nc.dram_tensor("k", [L, H], f32, kind="ExternalInput").ap()
    v_ap = nc.dram_tensor("v", [L, H], f32, kind="ExternalInput").ap()
    wq_ap = nc.dram_tensor("wq", [E, H], f32, kind="ExternalInput").ap()
    wk_ap = nc.dram_tensor("wk", [E, H], f32, kind="ExternalInput").ap()
    wv_ap = nc.dram_tensor("wv", [E, H], f32, kind="ExternalInput").ap()
    bq_ap = nc.dram_tensor("bq", [E], f32, kind="ExternalInput").ap()
    bk_ap = nc.dram_tensor("bk", [E], f32, kind="ExternalInput").ap()
    bv_ap = nc.dram_tensor("bv", [E], f32, kind="ExternalInput").ap()
    out_ap = nc.dram_tensor("out", [L, E], f32, kind="ExternalOutput").ap()

    with tile.TileContext(nc) as tc, ExitStack() as ctx:
        const = ctx.enter_context(tc.tile_pool(name="const", bufs=1))
        natp = ctx.enter_context(tc.tile_pool(name="nat", bufs=9))
        chp = ctx.enter_context(tc.tile_pool(name="ch", bufs=8))
        pTsb = ctx.enter_context(tc.tile_pool(name="pTsb", bufs=1))
        projps = ctx.enter_context(tc.tile_pool(name="projps", bufs=1, space="PSUM"))
        scps = ctx.enter_context(tc.tile_pool(name="scps", bufs=2, space="PSUM"))
        ktps = ctx.enter_context(tc.tile_pool(name="ktps", bufs=2, space="PSUM"))
        ptp = ctx.enter_context(tc.tile_pool(name="pt", bufs=3))
        ctxps = ctx.enter_context(tc.tile_pool(name="ctxps", bufs=2, space="PSUM"))
        tpsp = ctx.enter_context(tc.tile_pool(name="tps", bufs=1, space="PSUM"))
        epip = ctx.enter_context(tc.tile_pool(name="epi", bufs=4))

        # --- constants & weights: emitted via a deferred hook after the
        # first big loads so they don't block the Pool DMA queue; W is
        # sync-loaded f32 (HWDGE) and cast on VectorE, then xbar-transposed
        # to [128(h%128), 8(h//128), 64(e)] ---
        ident_f32 = const.tile([128, 128], f32)
        ident_bf16 = const.tile([128, 128], bf16)
        ident_fp8 = const.tile([128, 128], fp8)
        wtk8 = const.tile([128, HB, E], fp8)
        tri_f32 = const.tile([128, 128], f32)
        tri = const.tile([128, 128], bf16)
        wT = {}
        b_sb = {}
        wnatf = {}
        for _n in ("q", "k", "v"):
            wnatf[_n] = const.tile([E, H], f32, tag=f"wnatf_{_n}",
                                   name=f"wnatf_{_n}")
            wT[_n] = const.tile([128, HB, E], bf16, tag=f"wT_{_n}",
                                name=f"wT_{_n}")
            b_sb[_n] = const.tile([E, 1], f32, tag=f"b_{_n}",
                                  name=f"b_{_n}")

        w_xbars = []

        def emit_consts_and_weights(vaug):
            nc.vector.memset(vaug[:, :, E : E + 1], 1.0)
            make_identity(nc, ident_f32[:])
            nc.vector.tensor_copy(ident_bf16[:], ident_f32[:])
            nc.vector.tensor_copy(ident_fp8[:], ident_f32[:])
            # tri[r, c] = 1.0 where c >= r (valid: key row <= query col)
            make_upper_triangular(nc, tri_f32[:], val=1.0, diag=True)
            nc.vector.tensor_copy(tri[:], tri_f32[:])
            for name, w_ap, bias_ap in (
                ("q", wq_ap, bq_ap),
                ("k", wk_ap, bk_ap),
                ("v", wv_ap, bv_ap),
            ):
                nc.sync.dma_start(out=wnatf[name][:], in_=w_ap)
                wnat = const.tile([E, H], bf16, tag=f"wnat_{name}")
                nc.vector.tensor_copy(wnat[:], wnatf[name][:])
                w_xbars.append(
                    nc.sync.dma_start(out=wT[name][:], in_=wnat[:],
                                      transpose=True))
                nc.scalar.dma_start(out=b_sb[name][:], in_=bias_ap)
            nc.vector.tensor_copy(wtk8[:], wT["k"][:])

        # --- persistent projection outputs ---
        qpT = pTsb.tile([E, L], bf16, tag="qpT")
        kpT = pTsb.tile([E, L], bf16, tag="kpT")
        vpT = pTsb.tile([E, L], bf16, tag="vpT")
        vaug = pTsb.tile([128, NBLK, E + 1], bf16, tag="vaug")

        pT_of = {"q": qpT, "k": kpT, "v": vpT}
        x_ap_of = {"q": q_ap, "k": k_ap, "v": v_ap}

        out_dmas = []

        def emit_load(name, c):
            l0 = c * CHUNK
            # k is loaded in fp8: its quantization error only reaches the
            # softmax logits, which the 1/sqrt(L) scale compresses ~45x
            dtt = fp8 if name == "k" else bf16
            nat = natp.tile([128, TPC, H], dtt, tag="nat")
            src = x_ap_of[name][l0 : l0 + CHUNK, :].rearrange(
                "(t p) h -> p t h", p=128
            )
            # flat out AP: bigger contiguous runs -> half the SWDGE
            # descriptors, so more loads fit the descriptor ring at once
            ld = nc.gpsimd.dma_start(
                out=nat[:].rearrange("p t h -> p (t h)"), in_=src
            )  # f32 -> bf16 cast
            return nat, ld

        def emit_tp_and_proj(name, c, nat):
            l0 = c * CHUNK
            xb = None
            dtt = fp8 if name == "k" else bf16
            cht = chp.tile([128, TPC * H], dtt, tag="ch")
            if name == "k":
                # transpose on PE (saves serial-DMA xbar time): per (lt, hb)
                # 128x128 block transpose into PSUM, evacuate per-hb to SBUF
                # vT chunk [128, hb, l]; evac alternates ScalarE/VectorE.
                chv = cht[:].rearrange("p (hb l) -> p hb l", hb=HB, l=CHUNK)
                for hb in range(HB):
                    # fp8 transpose mode requires output element step 2
                    # (validated against the execution backend)
                    vt_ps = ktps.tile([128, 2 * CHUNK], fp8, tag="kt")
                    for t in range(TPC):
                        nc.tensor.transpose(
                            vt_ps[:, t * 256 : (t + 1) * 256 : 2],
                            nat[:, t, hb * 128 : (hb + 1) * 128],
                            ident_fp8[:],
                        )
                    vt_v = vt_ps[:, 0 : 2 * CHUNK : 2]
                    if hb % 2 == 1:
                        nc.scalar.activation(
                            chv[:, hb, :], vt_v, AF.Identity)
                    else:
                        nc.vector.tensor_copy(chv[:, hb, :], vt_v)
                rhs_of = lambda hb: chv[:, hb, :]
                w_st = wtk8
            else:
                # ONE xbar transpose per chunk: 3D out [128, TPC*HB, 128]
                # with out[a, b, c] = nat_flat[c, b*128 + a] (3D-out form
                # validated against the execution backend); free layout is
                # t*1024 + hb*128 + lp, so the projection's moving-operand AP
                # [(t: 1024), (lp: 1)] at offset hb*128 is natural l-order.
                chb = cht[:].rearrange(
                    "p (t hb lp) -> p t hb lp", t=TPC, hb=HB, lp=128
                )
                xb = nc.sync.dma_start(
                    out=cht[:].rearrange("p (b c) -> p b c", b=TPC * HB, c=128),
                    in_=nat[:].rearrange("p t h -> p (t h)"),
                    transpose=True,
                )
                rhs_of = lambda hb: chb[:, :, hb, :]
                w_st = wT[name]
            ps = projps.tile([E, CHUNK], f32, tag="projps")
            for hb in range(HB):
                nc.tensor.matmul(
                    ps[:],
                    lhsT=w_st[:, hb, :],
                    rhs=rhs_of(hb),
                    start=(hb == 0),
                    stop=(hb == HB - 1),
                )
            if name == "q":
                # VectorE is lighter-loaded than ScalarE here, and q's
                # projection gates each stripe's scores
                nc.vector.tensor_scalar_add(
                    pT_of[name][:, l0 : l0 + CHUNK], ps[:], b_sb[name][:])
            else:
                nc.scalar.activation(
                    pT_of[name][:, l0 : l0 + CHUNK], ps[:], AF.Identity,
                    bias=b_sb[name][:],
                )
            if name == "v":
                for t in range(TPC):
                    jb = c * TPC + t
                    vps = tpsp.tile([128, E + 1], bf16, tag="tps")
                    nc.tensor.transpose(
                        vps[:, :E],
                        vpT[:, jb * 128 : (jb + 1) * 128],
                        ident_bf16[:E, :E],
                    )
                    nc.vector.tensor_copy(vaug[:, jb, 0:E], vps[:, :E])
            return xb

        def begin_stripe(s):
            ctx_ps = ctxps.tile([E + 1, CHUNK], f32, tag="ctx")
            return {"s": s, "ctx": ctx_ps, "jmax": (s + 1) * TPC - 1}

        def emit_js(st, js):
            s, ctx_ps, jmax = st["s"], st["ctx"], st["jmax"]
            i0, i1 = s * CHUNK, (s + 1) * CHUNK
            # pair adjacent j's so exp runs on wider tiles (one PSUM bank)
            js = list(js)
            pairs = []
            while js:
                take = js[:1]
                w0 = i1 - max(i0, js[0] * 128)
                if len(js) > 1 and w0 + (i1 - max(i0, js[1] * 128)) <= 512:
                    take = js[:2]
                pairs.append(take)
                js = js[len(take):]
            def emit_ctx(pt, infos):
                for j, g0, w, o in infos:
                    if g0 == j * 128:  # diagonal block: causal mask
                        nc.vector.tensor_mul(
                            pt[:, o : o + 128], pt[:, o : o + 128], tri[:]
                        )
                    nc.tensor.matmul(
                        ctx_ps[:, g0 - i0 : g0 - i0 + w],
                        lhsT=vaug[:, j, :],
                        rhs=pt[:, o : o + w],
                        start=(j == 0),
                        stop=(j == jmax),
                    )

            # one-group software skew: PE's in-order queue sees
            # [scores_p, ctx_{p-1}] so it never stalls on exp_p
            pending = None
            for take in pairs:
                sc = scps.tile([128, 512], f32, tag="sc")
                pt = ptp.tile([128, 512], bf16, tag="pt")
                infos = []
                off = 0
                for j in take:
                    g0 = max(i0, j * 128)
                    w = i1 - g0
                    nc.tensor.matmul(
                        sc[:, off : off + w],
                        lhsT=kpT[:, j * 128 : (j + 1) * 128],
                        rhs=qpT[:, g0 : g0 + w],
                        start=True,
                        stop=True,
                    )
                    infos.append((j, g0, w, off))
                    off += w
                nc.scalar.activation(pt[:, 0:off], sc[:, 0:off], AF.Exp,
                                     scale=scale)
                if pending is not None:
                    emit_ctx(*pending)
                pending = (pt, infos)
            emit_ctx(*pending)

        def end_stripe(st):
            s, ctx_ps = st["s"], st["ctx"]
            i0, i1 = s * CHUNK, (s + 1) * CHUNK
            ctxsb = epip.tile([E + 1, CHUNK], f32, tag="ctxsb")
            nc.vector.tensor_copy(ctxsb[:], ctx_ps[:])
            outsb = epip.tile([128, TPC, E], f32, tag="outsb")
            for t in range(TPC):
                cps = tpsp.tile([128, E + 1], f32, tag="tps")
                nc.tensor.transpose(
                    cps[:],
                    ctxsb[:, t * 128 : (t + 1) * 128],
                    ident_f32[: E + 1, : E + 1],
                )
                rec = epip.tile([128, 1], f32, tag="rec")
                nc.vector.reciprocal(rec[:], cps[:, E : E + 1])
                nc.vector.tensor_scalar_mul(outsb[:, t, :], cps[:, 0:E], rec[:])
            dst = out_ap[i0:i1, :].rearrange("(t p) e -> p t e", p=128)
            out_dmas.append((dst, outsb))

        # Load order: (k,v,q) per chunk, q's last chunk hoisted before k/v's
        # last chunk; stripe s emitted once chunk s is fully emitted.
        load_order = []
        for c in range(NCHUNK - 1):
            load_order += [("k", c), ("v", c), ("q", c)]
        load_order += [("q", NCHUNK - 1), ("k", NCHUNK - 1), ("v", NCHUNK - 1)]

        consts_done = [False]
        for _ in range(reps):
            # Tile globally serializes every DMACopy<->DmaTranspose mode
            # transition (~2.5us dead DMA time each), so batch the stream
            # into phases: [6 loads][4 xbar transposes][6 loads][4 xbars]
            # [4 output copies] - 5 transitions instead of ~11.
            out_dmas.clear()
            # schedule: phase lists of loads; after each load's tp_proj,
            # run the stripe actions keyed to that (tensor, chunk).
            # Stripe-3's v-independent j's are emitted right after q3's
            # projection so they overlap the remaining loads; its
            # v3-dependent tail (j 12-15) comes after v3's vaug blocks.
            st_of = {}
            def s_begin(c):
                st_of[c] = begin_stripe(c)
            def s_js(c, js):
                emit_js(st_of[c], js)
            def s_end(c):
                end_stripe(st_of[c])
            phases = [
                (("k", 0), ("v", 0),
                 ("q", 0, lambda: (s_begin(0), s_js(0, range(4)), s_end(0))),
                 ("k", 1), ("v", 1),
                 ("q", 1, lambda: (s_begin(1), s_js(1, range(8)), s_end(1))),
                 ("q", 3, lambda: (s_begin(3), s_js(3, range(8))))),
                (("k", 2),
                 ("v", 2, lambda: s_js(3, range(8, 12))),
                 ("q", 2, lambda: (s_begin(2), s_js(2, range(12)), s_end(2))),
                 ("k", 3),
                 ("v", 3, lambda: (s_js(3, range(12, 16)), s_end(3)))),
            ]
            prev_last_xb = None
            for phase in phases:
                nats = []
                last_ld = None
                for item in phase:
                    n, c = item[0], item[1]
                    nat, ld = emit_load(n, c)
                    if prev_last_xb is not None:
                        add_dep_helper(
                            ld.ins, prev_last_xb.ins, sync=True,
                            reason="dma mode-phase grouping: loads after "
                                   "previous phase's transposes")
                    nats.append((item, nat))
                    last_ld = ld
                xbs = []
                if not consts_done[0]:
                    consts_done[0] = True
                    emit_consts_and_weights(vaug)
                    for wxb in w_xbars:
                        add_dep_helper(
                            wxb.ins, last_ld.ins, sync=True,
                            reason="dma mode-phase grouping: W transposes "
                                   "with first xbar group")
                for item, nat in nats:
                    n, c = item[0], item[1]
                    xb = emit_tp_and_proj(n, c, nat)
                    if xb is not None:
                        add_dep_helper(
                            xb.ins, last_ld.ins, sync=True,
                            reason="dma mode-phase grouping: transposes "
                                   "after all phase loads")
                        xbs.append(xb)
                    if len(item) > 2:
                        item[2]()
                if xbs:
                    prev_last_xb = xbs[-1]
            for dst, outsb in out_dmas:
                od = nc.scalar.dma_start(out=dst, in_=outsb[:])
                add_dep_helper(
                    od.ins, prev_last_xb.ins, sync=True,
                    reason="dma mode-phase grouping: outputs last")

    nc.compile()
    return nc


def _get_nc(reps=1):
    key = ("nc", reps)
    if key not in _CACHE:
        _CACHE[key] = _build_nc(reps)
    return _CACHE[key]


def kernel(q, k, v, key_padding_mask=None, Wq=None, bq=None, Wk=None, bk=None,
           Wv=None, bv=None):
    from concourse.bass_utils import run_bass_kernel_spmd

    nc = _get_nc()
    f = np.float32
    shared = {
        "wq": np.ascontiguousarray(Wq, dtype=f),
        "wk": np.ascontiguousarray(Wk, dtype=f),
        "wv": np.ascontiguousarray(Wv, dtype=f),
        "bq": np.ascontiguousarray(bq, dtype=f),
        "bk": np.ascontiguousarray(bk, dtype=f),
        "bv": np.ascontiguousarray(bv, dtype=f),
    }
    in_maps = []
    for n in range(NCORES):
        m = dict(shared)
        m["q"] = np.ascontiguousarray(q[n], dtype=f)
        m["k"] = np.ascontiguousarray(k[n], dtype=f)
        m["v"] = np.ascontiguousarray(v[n], dtype=f)
        in_maps.append(m)
    res = run_bass_kernel_spmd(nc, in_maps, core_ids=list(range(NCORES)))
    out = np.stack([res.results[i]["out"] for i in range(NCORES)], axis=0)
    return out.astype(np.float32)


# revision 75
# speedup vs baseline: 1.3631x; 1.0256x over previous
"""Causal attention layer (N=8, L=2048, H=1024, E=64) on 8 TRN2 NeuronCores.

Sharding: data-parallel over batch N — one batch element per core, Q/K/V
projection weights replicated. No collectives needed.

Per-core pipeline (memory-bound problem: 24MB of q/k/v per core):
  1. q/k/v cast-loaded (f32 DRAM -> bf16 SBUF, SWDGE cast DMA) in 512-row
     chunks, then ONE flat XBAR-DMA-transpose per (tensor, chunk):
     in [128, 4096] -> out [128, 4096] whose free index m encodes
     (lp, lt, hb) = (m//32, (m%32)//8, m%8); the projection's moving-operand
     APs read it with strides [(lt:8), (lp:32)] at offset hb, which restores
     natural l-order in PSUM columns.
  2. Projections: stationary WqT/WkT/WvT [128, 64] blocks (xbar-transposed
     once), moving chunk stripes -> qpT/kpT/vpT [64, 2048] bf16, bias added
     on ScalarE during the PSUM->SBUF copy.
  3. vpT is PE-transposed to natural vp [128, 65] blocks with an appended
     ones-column (makes the context matmul accumulate softmax row-sums for
     free).
  4. Scores computed transposed: PT[j, i] = exp(scale * kp_j . qp_i), exp on
     ScalarE with the 1/sqrt(L) scale folded in; causal mask = multiplicative
     upper-triangular mask on diagonal blocks (scores are tiny: no
     max-subtraction needed).
  5. ctxT[65, i] += vp_aug[j].T @ PT[j, i] accumulated over j in PSUM;
     epilogue PE-transposes ctxT back to natural, divides by the row-sum
     column, DMAs out per stripe.
Loads are emitted k,v,q per chunk with q's last chunk hoisted before k/v's
last chunk so the deep final attention stripe starts before the load stream
finishes.
"""

import math

import numpy as np

N, L, H, E = 8, 2048, 1024, 64
NCORES = 8
CHUNK = 512  # rows per load chunk
NCHUNK = L // CHUNK  # 4
TPC = CHUNK // 128  # 128-row tiles per chunk = 4
NBLK = L // 128  # 16 j/i blocks
HB = H // 128  # 8 h-blocks

_CACHE = {}


def _build_nc(reps=1):
    from contextlib import ExitStack

    import concourse.mybir as mybir
    import concourse.tile as tile
    from concourse import bacc
    from concourse.tile_rust import add_dep_helper
    from concourse.masks import make_identity, make_upper_triangular

    f32 = mybir.dt.float32
    bf16 = mybir.dt.bfloat16
    fp8 = mybir.dt.float8e4
    AF = mybir.ActivationFunctionType
    scale = 1.0 / math.sqrt(float(L))

    nc = bacc.Bacc("TRN2", target_bir_lowering=False, debug=False)

    q_ap = nc.dram_tensor("q", [L, H], f32, kind="ExternalInput").ap()
    k_ap = nc.dram_tensor("k", [L, H], f32, kind="ExternalInput").ap()
    v_ap = nc.dram_tensor("v", [L, H], f32, kind="ExternalInput").ap()
    wq_ap = nc.dram_tensor("wq", [E, H], f32, kind="ExternalInput").ap()
    wk_ap = nc.dram_tensor("wk", [E, H], f32, kind="ExternalInput").ap()
    wv_ap = nc.dram_tensor("wv", [E, H], f32, kind="ExternalInput").ap()
    bq_ap = nc.dram_tensor("bq", [E], f32, kind="ExternalInput").ap()
    bk_ap = nc.dram_tensor("bk", [E], f32, kind="ExternalInput").ap()
    bv_ap = nc.dram_tensor("bv", [E], f32, kind="ExternalInput").ap()
    out_ap = nc.dram_tensor("out", [L, E], f32, kind="ExternalOutput").ap()

    with tile.TileContext(nc) as tc, ExitStack() as ctx:
        const = ctx.enter_context(tc.tile_pool(name="const", bufs=1))
        natp = ctx.enter_context(tc.tile_pool(name="nat", bufs=9))
        chp = ctx.enter_context(tc.tile_pool(name="ch", bufs=8))
        pTsb = ctx.enter_context(tc.tile_pool(name="pTsb", bufs=1))
        projps = ctx.enter_context(tc.tile_pool(name="projps", bufs=1, space="PSUM"))
        scps = ctx.enter_context(tc.tile_pool(name="scps", bufs=2, space="PSUM"))
        ktps = ctx.enter_context(tc.tile_pool(name="ktps", bufs=2, space="PSUM"))
        ptp = ctx.enter_context(tc.tile_pool(name="pt", bufs=3))
        ctxps = ctx.enter_context(tc.tile_pool(name="ctxps", bufs=2, space="PSUM"))
        tpsp = ctx.enter_context(tc.tile_pool(name="tps", bufs=1, space="PSUM"))
        epip = ctx.enter_context(tc.tile_pool(name="epi", bufs=4))

        # --- constants & weights: emitted via a deferred hook after the
        # first big loads so they don't block the Pool DMA queue; W is
        # sync-loaded f32 (HWDGE) and cast on VectorE, then xbar-transposed
        # to [128(h%128), 8(h//128), 64(e)] ---
        ident_f32 = const.tile([128, 128], f32)
        ident_bf16 = const.tile([128, 128], bf16)
        ident_fp8 = const.tile([128, 128], fp8)
        wtk8 = const.tile([128, HB, E], fp8)
        tri_f32 = const.tile([128, 128], f32)
        tri = const.tile([128, 128], bf16)
        wT = {}
        b_sb = {}
        wnatf = {}
        for _n in ("q", "k", "v"):
            wnatf[_n] = const.tile([E, H], f32, tag=f"wnatf_{_n}",
                                   name=f"wnatf_{_n}")
            wT[_n] = const.tile([128, HB, E], bf16, tag=f"wT_{_n}",
                                name=f"wT_{_n}")
            b_sb[_n] = const.tile([E, 1], f32, tag=f"b_{_n}",
                                  name=f"b_{_n}")

        w_xbars = []

        def emit_consts_and_weights(vaug):
            nc.vector.memset(vaug[:, :, E : E + 1], 1.0)
            make_identity(nc, ident_f32[:])
            nc.vector.tensor_copy(ident_bf16[:], ident_f32[:])
            nc.vector.tensor_copy(ident_fp8[:], ident_f32[:])
            # tri[r, c] = 1.0 where c >= r (valid: key row <= query col)
            make_upper_triangular(nc, tri_f32[:], val=1.0, diag=True)
            nc.vector.tensor_copy(tri[:], tri_f32[:])
            for name, w_ap, bias_ap in (
                ("q", wq_ap, bq_ap),
                ("k", wk_ap, bk_ap),
                ("v", wv_ap, bv_ap),
            ):
                nc.sync.dma_start(out=wnatf[name][:], in_=w_ap)
                wnat = const.tile([E, H], bf16, tag=f"wnat_{name}")
                nc.vector.tensor_copy(wnat[:], wnatf[name][:])
                w_xbars.append(
                    nc.sync.dma_start(out=wT[name][:], in_=wnat[:],
                                      transpose=True))
                nc.scalar.dma_start(out=b_sb[name][:], in_=bias_ap)
            nc.vector.tensor_copy(wtk8[:], wT["k"][:])

        # --- persistent projection outputs ---
        qpT = pTsb.tile([E, L], bf16, tag="qpT")
        kpT = pTsb.tile([E, L], bf16, tag="kpT")
        vpT = pTsb.tile([E, L], bf16, tag="vpT")
        vaug = pTsb.tile([128, NBLK, E + 1], bf16, tag="vaug")

        pT_of = {"q": qpT, "k": kpT, "v": vpT}
        x_ap_of = {"q": q_ap, "k": k_ap, "v": v_ap}

        out_dmas = []

        def emit_load(name, c):
            l0 = c * CHUNK
            # k is loaded in fp8: its quantization error only reaches the
            # softmax logits, which the 1/sqrt(L) scale compresses ~45x
            dtt = fp8 if name == "k" else bf16
            nat = natp.tile([128, TPC, H], dtt, tag="nat")
            src = x_ap_of[name][l0 : l0 + CHUNK, :].rearrange(
                "(t p) h -> p t h", p=128
            )
            # flat out AP: bigger contiguous runs -> half the SWDGE
            # descriptors, so more loads fit the descriptor ring at once
            ld = nc.gpsimd.dma_start(
                out=nat[:].rearrange("p t h -> p (t h)"), in_=src
            )  # f32 -> bf16 cast
            return nat, ld

        def emit_tp_and_proj(name, c, nat):
            l0 = c * CHUNK
            xb = None
            dtt = fp8 if name == "k" else bf16
            cht = chp.tile([128, TPC * H], dtt, tag="ch")
            if name == "k":
                # transpose on PE (saves serial-DMA xbar time): per (lt, hb)
                # 128x128 block transpose into PSUM, evacuate per-hb to SBUF
                # vT chunk [128, hb, l]; evac alternates ScalarE/VectorE.
                chv = cht[:].rearrange("p (hb l) -> p hb l", hb=HB, l=CHUNK)
                for hb in range(HB):
                    # fp8 transpose mode requires output element step 2
                    # (validated against the execution backend)
                    vt_ps = ktps.tile([128, 2 * CHUNK], fp8, tag="kt")
                    for t in range(TPC):
                        nc.tensor.transpose(
                            vt_ps[:, t * 256 : (t + 1) * 256 : 2],
                            nat[:, t, hb * 128 : (hb + 1) * 128],
                            ident_fp8[:],
                        )
                    vt_v = vt_ps[:, 0 : 2 * CHUNK : 2]
                    if hb % 2 == 1:
                        nc.scalar.activation(
                            chv[:, hb, :], vt_v, AF.Identity)
                    else:
                        nc.vector.tensor_copy(chv[:, hb, :], vt_v)
                rhs_of = lambda hb: chv[:, hb, :]
                w_st = wtk8
            else:
                # ONE xbar transpose per chunk: 3D out [128, TPC*HB, 128]
                # with out[a, b, c] = nat_flat[c, b*128 + a] (3D-out form
                # validated against the execution backend); free layout is
                # t*1024 + hb*128 + lp, so the projection's moving-operand AP
                # [(t: 1024), (lp: 1)] at offset hb*128 is natural l-order.
                chb = cht[:].rearrange(
                    "p (t hb lp) -> p t hb lp", t=TPC, hb=HB, lp=128
                )
                xb = nc.sync.dma_start(
                    out=cht[:].rearrange("p (b c) -> p b c", b=TPC * HB, c=128),
                    in_=nat[:].rearrange("p t h -> p (t h)"),
                    transpose=True,
                )
                rhs_of = lambda hb: chb[:, :, hb, :]
                w_st = wT[name]
            ps = projps.tile([E, CHUNK], f32, tag="projps")
            for hb in range(HB):
                nc.tensor.matmul(
                    ps[:],
                    lhsT=w_st[:, hb, :],
                    rhs=rhs_of(hb),
                    start=(hb == 0),
                    stop=(hb == HB - 1),
                )
            if name == "q":
                # VectorE is lighter-loaded than ScalarE here, and q's
                # projection gates each stripe's scores
                nc.vector.tensor_scalar_add(
                    pT_of[name][:, l0 : l0 + CHUNK], ps[:], b_sb[name][:])
            else:
                nc.scalar.activation(
                    pT_of[name][:, l0 : l0 + CHUNK], ps[:], AF.Identity,
                    bias=b_sb[name][:],
                )
            if name == "v":
                for t in range(TPC):
                    jb = c * TPC + t
                    vps = tpsp.tile([128, E + 1], bf16, tag="tps")
                    nc.tensor.transpose(
                        vps[:, :E],
                        vpT[:, jb * 128 : (jb + 1) * 128],
                        ident_bf16[:E, :E],
                    )
                    nc.vector.tensor_copy(vaug[:, jb, 0:E], vps[:, :E])
            return xb

        def begin_stripe(s):
            ctx_ps = ctxps.tile([E + 1, CHUNK], f32, tag="ctx")
            return {"s": s, "ctx": ctx_ps, "jmax": (s + 1) * TPC - 1}

        def emit_js(st, js):
            s, ctx_ps, jmax = st["s"], st["ctx"], st["jmax"]
            i0, i1 = s * CHUNK, (s + 1) * CHUNK
            # pair adjacent j's so exp runs on wider tiles (one PSUM bank)
            js = list(js)
            pairs = []
            while js:
                take = js[:1]
                w0 = i1 - max(i0, js[0] * 128)
                if len(js) > 1 and w0 + (i1 - max(i0, js[1] * 128)) <= 512:
                    take = js[:2]
                pairs.append(take)
                js = js[len(take):]
            def emit_ctx(pt, infos):
                for j, g0, w, o in infos:
                    if g0 == j * 128:  # diagonal block: causal mask
                        nc.vector.tensor_mul(
                            pt[:, o : o + 128], pt[:, o : o + 128], tri[:]
                        )
                    nc.tensor.matmul(
                        ctx_ps[:, g0 - i0 : g0 - i0 + w],
                        lhsT=vaug[:, j, :],
                        rhs=pt[:, o : o + w],
                        start=(j == 0),
                        stop=(j == jmax),
                    )

            # one-group software skew: PE's in-order queue sees
            # [scores_p, ctx_{p-1}] so it never stalls on exp_p
            pending = None
            for take in pairs:
                sc = scps.tile([128, 512], f32, tag="sc")
                pt = ptp.tile([128, 512], bf16, tag="pt")
                infos = []
                off = 0
                for j in take:
                    g0 = max(i0, j * 128)
                    w = i1 - g0
                    nc.tensor.matmul(
                        sc[:, off : off + w],
                        lhsT=kpT[:, j * 128 : (j + 1) * 128],
                        rhs=qpT[:, g0 : g0 + w],
                        start=True,
                        stop=True,
                    )
                    infos.append((j, g0, w, off))
                    off += w
                nc.scalar.activation(pt[:, 0:off], sc[:, 0:off], AF.Exp,
                                     scale=scale)
                if pending is not None:
                    emit_ctx(*pending)
                pending = (pt, infos)
            emit_ctx(*pending)

        def end_stripe(st):
            s, ctx_ps = st["s"], st["ctx"]
            i0, i1 = s * CHUNK, (s + 1) * CHUNK
            ctxsb = epip.tile([E + 1, CHUNK], f32, tag="ctxsb")
            nc.vector.tensor_copy(ctxsb[:], ctx_ps[:])
            outsb = epip.tile([128, TPC, E], f32, tag="outsb")
            for t in range(TPC):
                cps = tpsp.tile([128, E + 1], f32, tag="tps")
                nc.tensor.transpose(
                    cps[:],
                    ctxsb[:, t * 128 : (t + 1) * 128],
                    ident_f32[: E + 1, : E + 1],
                )
                rec = epip.tile([128, 1], f32, tag="rec")
                nc.vector.reciprocal(rec[:], cps[:, E : E + 1])
                nc.vector.tensor_scalar_mul(outsb[:, t, :], cps[:, 0:E], rec[:])
            dst = out_ap[i0:i1, :].rearrange("(t p) e -> p t e", p=128)
            out_dmas.append((dst, outsb))

        # Load order: (k,v,q) per chunk, q's last chunk hoisted before k/v's
        # last chunk; stripe s emitted once chunk s is fully emitted.
        load_order = []
        for c in range(NCHUNK - 1):
            load_order += [("k", c), ("v", c), ("q", c)]
        load_order += [("q", NCHUNK - 1), ("k", NCHUNK - 1), ("v", NCHUNK - 1)]

        consts_done = [False]
        for _ in range(reps):
            # Tile globally serializes every DMACopy<->DmaTranspose mode
            # transition (~2.5us dead DMA time each), so batch the stream
            # into phases: [6 loads][4 xbar transposes][6 loads][4 xbars]
            # [4 output copies] - 5 transitions instead of ~11.
            out_dmas.clear()
            # schedule: phase lists of loads; after each load's tp_proj,
            # run the stripe actions keyed to that (tensor, chunk).
            # Stripe-3's v-independent j's are emitted right after q3's
            # projection so they overlap the remaining loads; its
            # v3-dependent tail (j 12-15) comes after v3's vaug blocks.
            st_of = {}
            def s_begin(c):
                st_of[c] = begin_stripe(c)
            def s_js(c, js):
                emit_js(st_of[c], js)
            def s_end(c):
                end_stripe(st_of[c])
            phases = [
                (("k", 0), ("v", 0),
                 ("q", 0, lambda: (s_begin(0), s_js(0, range(4)), s_end(0))),
                 ("k", 1), ("v", 1),
                 ("q", 1, lambda: (s_begin(1), s_js(1, range(8)), s_end(1))),
                 ("q", 3, lambda: (s_begin(3), s_js(3, range(8)))),
                 ("q", 2, lambda: (s_begin(2), s_js(2, range(8))))),
                (("k", 2),
                 ("v", 2, lambda: (s_js(3, range(8, 12)),
                                   s_js(2, range(8, 12)), s_end(2))),
                 ("k", 3),
                 ("v", 3, lambda: (s_js(3, range(12, 16)), s_end(3)))),
            ]
            prev_last_xb = None
            for phase in phases:
                nats = []
                last_ld = None
                for item in phase:
                    n, c = item[0], item[1]
                    nat, ld = emit_load(n, c)
                    if prev_last_xb is not None:
                        add_dep_helper(
                            ld.ins, prev_last_xb.ins, sync=True,
                            reason="dma mode-phase grouping: loads after "
                                   "previous phase's transposes")
                    nats.append((item, nat))
                    last_ld = ld
                xbs = []
                if not consts_done[0]:
                    consts_done[0] = True
                    emit_consts_and_weights(vaug)
                    for wxb in w_xbars:
                        add_dep_helper(
                            wxb.ins, last_ld.ins, sync=True,
                            reason="dma mode-phase grouping: W transposes "
                                   "with first xbar group")
                for item, nat in nats:
                    n, c = item[0], item[1]
                    xb = emit_tp_and_proj(n, c, nat)
                    if xb is not None:
                        add_dep_helper(
                            xb.ins, last_ld.ins, sync=True,
                            reason="dma mode-phase grouping: transposes "
                                   "after all phase loads")
                        xbs.append(xb)
                    if len(item) > 2:
                        item[2]()
                if xbs:
                    prev_last_xb = xbs[-1]
            for dst, outsb in out_dmas:
                od = nc.scalar.dma_start(out=dst, in_=outsb[:])
                add_dep_helper(
                    od.ins, prev_last_xb.ins, sync=True,
                    reason="dma mode-phase grouping: outputs last")

    nc.compile()
    return nc


def _get_nc(reps=1):
    key = ("nc", reps)
    if key not in _CACHE:
        _CACHE[key] = _build_nc(reps)
    return _CACHE[key]


def kernel(q, k, v, key_padding_mask=None, Wq=None, bq=None, Wk=None, bk=None,
           Wv=None, bv=None):
    from concourse.bass_utils import run_bass_kernel_spmd

    nc = _get_nc()
    f = np.float32
    shared = {
        "wq": np.ascontiguousarray(Wq, dtype=f),
        "wk": np.ascontiguousarray(Wk, dtype=f),
        "wv": np.ascontiguousarray(Wv, dtype=f),
        "bq": np.ascontiguousarray(bq, dtype=f),
        "bk": np.ascontiguousarray(bk, dtype=f),
        "bv": np.ascontiguousarray(bv, dtype=f),
    }
    in_maps = []
    for n in range(NCORES):
        m = dict(shared)
        m["q"] = np.ascontiguousarray(q[n], dtype=f)
        m["k"] = np.ascontiguousarray(k[n], dtype=f)
        m["v"] = np.ascontiguousarray(v[n], dtype=f)
        in_maps.append(m)
    res = run_bass_kernel_spmd(nc, in_maps, core_ids=list(range(NCORES)))
    out = np.stack([res.results[i]["out"] for i in range(NCORES)], axis=0)
    return out.astype(np.float32)


# revision 83
# speedup vs baseline: 1.3709x; 1.0058x over previous
"""Causal attention layer (N=8, L=2048, H=1024, E=64) on 8 TRN2 NeuronCores.

Sharding: data-parallel over batch N — one batch element per core, Q/K/V
projection weights replicated. No collectives needed.

Per-core pipeline (memory-bound problem: 24MB of q/k/v per core):
  1. q/k/v cast-loaded (f32 DRAM -> bf16 SBUF, SWDGE cast DMA) in 512-row
     chunks, then ONE flat XBAR-DMA-transpose per (tensor, chunk):
     in [128, 4096] -> out [128, 4096] whose free index m encodes
     (lp, lt, hb) = (m//32, (m%32)//8, m%8); the projection's moving-operand
     APs read it with strides [(lt:8), (lp:32)] at offset hb, which restores
     natural l-order in PSUM columns.
  2. Projections: stationary WqT/WkT/WvT [128, 64] blocks (xbar-transposed
     once), moving chunk stripes -> qpT/kpT/vpT [64, 2048] bf16, bias added
     on ScalarE during the PSUM->SBUF copy.
  3. vpT is PE-transposed to natural vp [128, 65] blocks with an appended
     ones-column (makes the context matmul accumulate softmax row-sums for
     free).
  4. Scores computed transposed: PT[j, i] = exp(scale * kp_j . qp_i), exp on
     ScalarE with the 1/sqrt(L) scale folded in; causal mask = multiplicative
     upper-triangular mask on diagonal blocks (scores are tiny: no
     max-subtraction needed).
  5. ctxT[65, i] += vp_aug[j].T @ PT[j, i] accumulated over j in PSUM;
     epilogue PE-transposes ctxT back to natural, divides by the row-sum
     column, DMAs out per stripe.
Loads are emitted k,v,q per chunk with q's last chunk hoisted before k/v's
last chunk so the deep final attention stripe starts before the load stream
finishes.
"""

import math

import numpy as np

N, L, H, E = 8, 2048, 1024, 64
NCORES = 8
CHUNK = 512  # rows per load chunk
NCHUNK = L // CHUNK  # 4
TPC = CHUNK // 128  # 128-row tiles per chunk = 4
NBLK = L // 128  # 16 j/i blocks
HB = H // 128  # 8 h-blocks

_CACHE = {}


def _build_nc(reps=1):
    from contextlib import ExitStack

    import concourse.mybir as mybir
    import concourse.tile as tile
    from concourse import bacc
    from concourse.tile_rust import add_dep_helper
    from concourse.masks import make_identity, make_upper_triangular

    f32 = mybir.dt.float32
    bf16 = mybir.dt.bfloat16
    fp8 = mybir.dt.float8e4
    AF = mybir.ActivationFunctionType
    scale = 1.0 / math.sqrt(float(L))

    nc = bacc.Bacc("TRN2", target_bir_lowering=False, debug=False)

    q_ap = nc.dram_tensor("q", [L, H], f32, kind="ExternalInput").ap()
    k_ap = nc.dram_tensor("k", [L, H], f32, kind="ExternalInput").ap()
    v_ap = nc.dram_tensor("v", [L, H], f32, kind="ExternalInput").ap()
    wq_ap = nc.dram_tensor("wq", [E, H], f32, kind="ExternalInput").ap()
    wk_ap = nc.dram_tensor("wk", [E, H], f32, kind="ExternalInput").ap()
    wv_ap = nc.dram_tensor("wv", [E, H], f32, kind="ExternalInput").ap()
    bq_ap = nc.dram_tensor("bq", [E], f32, kind="ExternalInput").ap()
    bk_ap = nc.dram_tensor("bk", [E], f32, kind="ExternalInput").ap()
    bv_ap = nc.dram_tensor("bv", [E], f32, kind="ExternalInput").ap()
    out_ap = nc.dram_tensor("out", [L, E], f32, kind="ExternalOutput").ap()

    with tile.TileContext(nc) as tc, ExitStack() as ctx:
        const = ctx.enter_context(tc.tile_pool(name="const", bufs=1))
        natp = ctx.enter_context(tc.tile_pool(name="nat", bufs=9))
        chp = ctx.enter_context(tc.tile_pool(name="ch", bufs=8))
        pTsb = ctx.enter_context(tc.tile_pool(name="pTsb", bufs=1))
        projps = ctx.enter_context(tc.tile_pool(name="projps", bufs=1, space="PSUM"))
        scps = ctx.enter_context(tc.tile_pool(name="scps", bufs=2, space="PSUM"))
        ktps = ctx.enter_context(tc.tile_pool(name="ktps", bufs=2, space="PSUM"))
        ptp = ctx.enter_context(tc.tile_pool(name="pt", bufs=3))
        ctxps = ctx.enter_context(tc.tile_pool(name="ctxps", bufs=2, space="PSUM"))
        tpsp = ctx.enter_context(tc.tile_pool(name="tps", bufs=1, space="PSUM"))
        epip = ctx.enter_context(tc.tile_pool(name="epi", bufs=4))

        # --- constants & weights: emitted via a deferred hook after the
        # first big loads so they don't block the Pool DMA queue; W is
        # sync-loaded f32 (HWDGE) and cast on VectorE, then xbar-transposed
        # to [128(h%128), 8(h//128), 64(e)] ---
        ident_f32 = const.tile([128, 128], f32)
        ident_bf16 = const.tile([128, 128], bf16)
        ident_fp8 = const.tile([128, 128], fp8)
        wtk8 = const.tile([128, HB, E], fp8)
        wtq8 = const.tile([128, HB, E], fp8)
        tri_f32 = const.tile([128, 128], f32)
        tri = const.tile([128, 128], bf16)
        wT = {}
        b_sb = {}
        wnatf = {}
        for _n in ("q", "k", "v"):
            wnatf[_n] = const.tile([E, H], f32, tag=f"wnatf_{_n}",
                                   name=f"wnatf_{_n}")
            wT[_n] = const.tile([128, HB, E], bf16, tag=f"wT_{_n}",
                                name=f"wT_{_n}")
            b_sb[_n] = const.tile([E, 1], f32, tag=f"b_{_n}",
                                  name=f"b_{_n}")

        w_xbars = []

        def emit_consts_and_weights(vaug):
            nc.vector.memset(vaug[:, :, E : E + 1], 1.0)
            make_identity(nc, ident_f32[:])
            nc.vector.tensor_copy(ident_bf16[:], ident_f32[:])
            nc.vector.tensor_copy(ident_fp8[:], ident_f32[:])
            # tri[r, c] = 1.0 where c >= r (valid: key row <= query col)
            make_upper_triangular(nc, tri_f32[:], val=1.0, diag=True)
            nc.vector.tensor_copy(tri[:], tri_f32[:])
            for name, w_ap, bias_ap in (
                ("q", wq_ap, bq_ap),
                ("k", wk_ap, bk_ap),
                ("v", wv_ap, bv_ap),
            ):
                nc.sync.dma_start(out=wnatf[name][:], in_=w_ap)
                wnat = const.tile([E, H], bf16, tag=f"wnat_{name}")
                nc.vector.tensor_copy(wnat[:], wnatf[name][:])
                w_xbars.append(
                    nc.sync.dma_start(out=wT[name][:], in_=wnat[:],
                                      transpose=True))
                nc.scalar.dma_start(out=b_sb[name][:], in_=bias_ap)
            nc.vector.tensor_copy(wtk8[:], wT["k"][:])
            nc.vector.tensor_copy(wtq8[:], wT["q"][:])

        # --- persistent projection outputs ---
        qpT = pTsb.tile([E, L], bf16, tag="qpT")
        kpT = pTsb.tile([E, L], bf16, tag="kpT")
        vpT = pTsb.tile([E, L], bf16, tag="vpT")
        vaug = pTsb.tile([128, NBLK, E + 1], bf16, tag="vaug")

        pT_of = {"q": qpT, "k": kpT, "v": vpT}
        x_ap_of = {"q": q_ap, "k": k_ap, "v": v_ap}

        out_dmas = []

        def emit_load(name, c):
            l0 = c * CHUNK
            # q and k are loaded in fp8: their quantization error only
            # reaches the softmax logits, which the 1/sqrt(L) scale
            # compresses ~45x
            dtt = fp8 if name == "k" or (name == "q" and c >= 2) else bf16
            nat = natp.tile([128, TPC, H], dtt, tag="nat")
            src = x_ap_of[name][l0 : l0 + CHUNK, :].rearrange(
                "(t p) h -> p t h", p=128
            )
            # flat out AP: bigger contiguous runs -> half the SWDGE
            # descriptors, so more loads fit the descriptor ring at once
            ld = nc.gpsimd.dma_start(
                out=nat[:].rearrange("p t h -> p (t h)"), in_=src
            )  # f32 -> bf16 cast
            return nat, ld

        def emit_tp_and_proj(name, c, nat):
            l0 = c * CHUNK
            xb = None
            pe_path = name == "k" or (name == "q" and c >= 2)
            dtt = fp8 if pe_path else bf16
            cht = chp.tile([128, TPC * H], dtt, tag="ch")
            if pe_path:
                # transpose on PE (saves serial-DMA xbar time): per (lt, hb)
                # 128x128 block transpose into PSUM, evacuate per-hb to SBUF
                # vT chunk [128, hb, l]; evac alternates ScalarE/VectorE.
                chv = cht[:].rearrange("p (hb l) -> p hb l", hb=HB, l=CHUNK)
                for hb in range(HB):
                    # fp8 transpose mode requires output element step 2
                    # (validated against the execution backend)
                    vt_ps = ktps.tile([128, 2 * CHUNK], fp8, tag="kt")
                    for t in range(TPC):
                        nc.tensor.transpose(
                            vt_ps[:, t * 256 : (t + 1) * 256 : 2],
                            nat[:, t, hb * 128 : (hb + 1) * 128],
                            ident_fp8[:],
                        )
                    vt_v = vt_ps[:, 0 : 2 * CHUNK : 2]
                    if hb % 2 == 1:
                        nc.scalar.activation(
                            chv[:, hb, :], vt_v, AF.Identity)
                    else:
                        nc.vector.tensor_copy(chv[:, hb, :], vt_v)
                rhs_of = lambda hb: chv[:, hb, :]
                w_st = wtk8 if name == "k" else wtq8
            else:
                # ONE xbar transpose per chunk: 3D out [128, TPC*HB, 128]
                # with out[a, b, c] = nat_flat[c, b*128 + a] (3D-out form
                # validated against the execution backend); free layout is
                # t*1024 + hb*128 + lp, so the projection's moving-operand AP
                # [(t: 1024), (lp: 1)] at offset hb*128 is natural l-order.
                chb = cht[:].rearrange(
                    "p (t hb lp) -> p t hb lp", t=TPC, hb=HB, lp=128
                )
                xb = nc.sync.dma_start(
                    out=cht[:].rearrange("p (b c) -> p b c", b=TPC * HB, c=128),
                    in_=nat[:].rearrange("p t h -> p (t h)"),
                    transpose=True,
                )
                rhs_of = lambda hb: chb[:, :, hb, :]
                w_st = wT[name]
            ps = projps.tile([E, CHUNK], f32, tag="projps")
            for hb in range(HB):
                nc.tensor.matmul(
                    ps[:],
                    lhsT=w_st[:, hb, :],
                    rhs=rhs_of(hb),
                    start=(hb == 0),
                    stop=(hb == HB - 1),
                )
            if name == "q":
                # VectorE is lighter-loaded than ScalarE here, and q's
                # projection gates each stripe's scores
                nc.vector.tensor_scalar_add(
                    pT_of[name][:, l0 : l0 + CHUNK], ps[:], b_sb[name][:])
            else:
                nc.scalar.activation(
                    pT_of[name][:, l0 : l0 + CHUNK], ps[:], AF.Identity,
                    bias=b_sb[name][:],
                )
            if name == "v":
                for t in range(TPC):
                    jb = c * TPC + t
                    vps = tpsp.tile([128, E + 1], bf16, tag="tps")
                    nc.tensor.transpose(
                        vps[:, :E],
                        vpT[:, jb * 128 : (jb + 1) * 128],
                        ident_bf16[:E, :E],
                    )
                    nc.vector.tensor_copy(vaug[:, jb, 0:E], vps[:, :E])
            return xb

        def begin_stripe(s):
            ctx_ps = ctxps.tile([E + 1, CHUNK], f32, tag="ctx")
            return {"s": s, "ctx": ctx_ps, "jmax": (s + 1) * TPC - 1}

        def emit_js(st, js):
            s, ctx_ps, jmax = st["s"], st["ctx"], st["jmax"]
            i0, i1 = s * CHUNK, (s + 1) * CHUNK
            # pair adjacent j's so exp runs on wider tiles (one PSUM bank)
            js = list(js)
            pairs = []
            while js:
                take = js[:1]
                w0 = i1 - max(i0, js[0] * 128)
                if len(js) > 1 and w0 + (i1 - max(i0, js[1] * 128)) <= 512:
                    take = js[:2]
                pairs.append(take)
                js = js[len(take):]
            def emit_ctx(pt, infos):
                for j, g0, w, o in infos:
                    if g0 == j * 128:  # diagonal block: causal mask
                        nc.vector.tensor_mul(
                            pt[:, o : o + 128], pt[:, o : o + 128], tri[:]
                        )
                    nc.tensor.matmul(
                        ctx_ps[:, g0 - i0 : g0 - i0 + w],
                        lhsT=vaug[:, j, :],
                        rhs=pt[:, o : o + w],
                        start=(j == 0),
                        stop=(j == jmax),
                    )

            # one-group software skew: PE's in-order queue sees
            # [scores_p, ctx_{p-1}] so it never stalls on exp_p
            pending = None
            for take in pairs:
                sc = scps.tile([128, 512], f32, tag="sc")
                pt = ptp.tile([128, 512], bf16, tag="pt")
                infos = []
                off = 0
                for j in take:
                    g0 = max(i0, j * 128)
                    w = i1 - g0
                    nc.tensor.matmul(
                        sc[:, off : off + w],
                        lhsT=kpT[:, j * 128 : (j + 1) * 128],
                        rhs=qpT[:, g0 : g0 + w],
                        start=True,
                        stop=True,
                    )
                    infos.append((j, g0, w, off))
                    off += w
                nc.scalar.activation(pt[:, 0:off], sc[:, 0:off], AF.Exp,
                                     scale=scale)
                if pending is not None:
                    emit_ctx(*pending)
                pending = (pt, infos)
            emit_ctx(*pending)

        def end_stripe(st):
            s, ctx_ps = st["s"], st["ctx"]
            i0, i1 = s * CHUNK, (s + 1) * CHUNK
            ctxsb = epip.tile([E + 1, CHUNK], f32, tag="ctxsb")
            nc.vector.tensor_copy(ctxsb[:], ctx_ps[:])
            outsb = epip.tile([128, TPC, E], f32, tag="outsb")
            for t in range(TPC):
                cps = tpsp.tile([128, E + 1], f32, tag="tps")
                nc.tensor.transpose(
                    cps[:],
                    ctxsb[:, t * 128 : (t + 1) * 128],
                    ident_f32[: E + 1, : E + 1],
                )
                rec = epip.tile([128, 1], f32, tag="rec")
                nc.vector.reciprocal(rec[:], cps[:, E : E + 1])
                nc.vector.tensor_scalar_mul(outsb[:, t, :], cps[:, 0:E], rec[:])
            dst = out_ap[i0:i1, :].rearrange("(t p) e -> p t e", p=128)
            out_dmas.append((dst, outsb))

        # Load order: (k,v,q) per chunk, q's last chunk hoisted before k/v's
        # last chunk; stripe s emitted once chunk s is fully emitted.
        load_order = []
        for c in range(NCHUNK - 1):
            load_order += [("k", c), ("v", c), ("q", c)]
        load_order += [("q", NCHUNK - 1), ("k", NCHUNK - 1), ("v", NCHUNK - 1)]

        consts_done = [False]
        for _ in range(reps):
            # Tile globally serializes every DMACopy<->DmaTranspose mode
            # transition (~2.5us dead DMA time each), so batch the stream
            # into phases: [6 loads][4 xbar transposes][6 loads][4 xbars]
            # [4 output copies] - 5 transitions instead of ~11.
            out_dmas.clear()
            # schedule: phase lists of loads; after each load's tp_proj,
            # run the stripe actions keyed to that (tensor, chunk).
            # Stripe-3's v-independent j's are emitted right after q3's
            # projection so they overlap the remaining loads; its
            # v3-dependent tail (j 12-15) comes after v3's vaug blocks.
            st_of = {}
            def s_begin(c):
                st_of[c] = begin_stripe(c)
            def s_js(c, js):
                emit_js(st_of[c], js)
            def s_end(c):
                end_stripe(st_of[c])
            phases = [
                (("k", 0), ("v", 0),
                 ("q", 0, lambda: (s_begin(0), s_js(0, range(4)), s_end(0))),
                 ("k", 1), ("v", 1),
                 ("q", 1, lambda: (s_begin(1), s_js(1, range(8)), s_end(1))),
                 ("q", 3, lambda: (s_begin(3), s_js(3, range(8)))),
                 ("q", 2, lambda: (s_begin(2), s_js(2, range(8))))),
                (("k", 2),
                 ("v", 2, lambda: (s_js(3, range(8, 12)),
                                   s_js(2, range(8, 12)), s_end(2))),
                 ("k", 3),
                 ("v", 3, lambda: (s_js(3, range(12, 16)), s_end(3)))),
            ]
            prev_last_xb = None
            for phase in phases:
                nats = []
                last_ld = None
                for item in phase:
                    n, c = item[0], item[1]
                    nat, ld = emit_load(n, c)
                    if prev_last_xb is not None:
                        add_dep_helper(
                            ld.ins, prev_last_xb.ins, sync=True,
                            reason="dma mode-phase grouping: loads after "
                                   "previous phase's transposes")
                    nats.append((item, nat))
                    last_ld = ld
                xbs = []
                if not consts_done[0]:
                    consts_done[0] = True
                    emit_consts_and_weights(vaug)
                    for wxb in w_xbars:
                        add_dep_helper(
                            wxb.ins, last_ld.ins, sync=True,
                            reason="dma mode-phase grouping: W transposes "
                                   "with first xbar group")
                for item, nat in nats:
                    n, c = item[0], item[1]
                    xb = emit_tp_and_proj(n, c, nat)
                    if xb is not None:
                        add_dep_helper(
                            xb.ins, last_ld.ins, sync=True,
                            reason="dma mode-phase grouping: transposes "
                                   "after all phase loads")
                        xbs.append(xb)
                    if len(item) > 2:
                        item[2]()
                if xbs:
                    prev_last_xb = xbs[-1]
            for dst, outsb in out_dmas:
                od = nc.scalar.dma_start(out=dst, in_=outsb[:])
                add_dep_helper(
                    od.ins, prev_last_xb.ins, sync=True,
                    reason="dma mode-phase grouping: outputs last")

    nc.compile()
    return nc


def _get_nc(reps=1):
    key = ("nc", reps)
    if key not in _CACHE:
        _CACHE[key] = _build_nc(reps)
    return _CACHE[key]


def kernel(q, k, v, key_padding_mask=None, Wq=None, bq=None, Wk=None, bk=None,
           Wv=None, bv=None):
    from concourse.bass_utils import run_bass_kernel_spmd

    nc = _get_nc()
    f = np.float32
    shared = {
        "wq": np.ascontiguousarray(Wq, dtype=f),
        "wk": np.ascontiguousarray(Wk, dtype=f),
        "wv": np.ascontiguousarray(Wv, dtype=f),
        "bq": np.ascontiguousarray(bq, dtype=f),
        "bk": np.ascontiguousarray(bk, dtype=f),
        "bv": np.ascontiguousarray(bv, dtype=f),
    }
    in_maps = []
    for n in range(NCORES):
        m = dict(shared)
        m["q"] = np.ascontiguousarray(q[n], dtype=f)
        m["k"] = np.ascontiguousarray(k[n], dtype=f)
        m["v"] = np.ascontiguousarray(v[n], dtype=f)
        in_maps.append(m)
    res = run_bass_kernel_spmd(nc, in_maps, core_ids=list(range(NCORES)))
    out = np.stack([res.results[i]["out"] for i in range(NCORES)], axis=0)
    return out.astype(np.float32)
